# revision 1
# baseline (speedup 1.0000x reference)
"""BasicTransformerBlock on 8 TRN2 NeuronCores.

Sharding: sequence-parallel, zero collectives. The [B=2, N=2048, D=512]
residual stream is split into 8 row-blocks of 512 (4 cores per batch
element). Every core recomputes the cheap batch-wide work it needs
(adaln1 + K/V projections over its batch's 2048 rows, cond K/V), and does
attention / FFN only for its own 512 query rows.

Per-core inputs are pre-rotated with np.roll so that "own" rows are always
rows 0:512 -- the SPMD program is identical on all cores, only data differs.
Attention is permutation-invariant over keys, so rolled K/V is fine.

Layouts (SBUF tiles are [128 partitions, ...]):
  hT  = normed activations, transposed: [128 p=d%128, 4 dtile, rows] bf16
  kT  = [128 p=dout%128, 4 dtile, rows] bf16   (head pair 2t,2t+1 stacked
        in partitions 0:64 / 64:128 of dtile t)
  qz  = [128, 4 ht, 2 slot, rows] bf16: slot s holds head 2ht+s in its own
        64-partition half, the other half ZERO.
  vC  = [128 p=row%128, njt*520+63] bf16 flat tile: per key tile jt, eight
        contiguous 65-col head windows [v_h(64) | 1]. Head h's av weight
        window is cols [jt*520+65h : +128] = [v_h | 1 | v_{h+1}[0:63]] --
        a full M=128 window with real data, no extra copies. The ones
        column makes the attention-weight row-sum (softmax denominator)
        fall out of the same matmul that computes attn@v; the overrun
        columns produce garbage psum rows 65:127 that are never read.

Every attention matmul is a full 128x128-array op (scores: K=128 via the
stacked head pair against a zero-padded q half; attn@v: M=128 via the
overlapped v windows). The PE HAM activity monitor only counts full-array
matmuls as "busy": half-array ops (K=64 or M=65) leave the clock gate at
K=4/8 (1.2 GHz effective) for the whole attention phase, which is where
the previous version lost ~120us.

Scores are computed transposed, sT[j, i], so exp() runs on ScalarE straight
out of PSUM and softmax normalization is applied per head on the tiny
attn@v result. Matmul operands are bf16 (weights are cast during the DMA
load by SWDGE); all accumulation/psum/residual math stays fp32.
"""

import contextlib

import numpy as np

import concourse.bass as bass
import concourse.mybir as mybir
import concourse.tile as tile
from concourse import bacc
from concourse.bass_utils import run_bass_kernel_spmd
from concourse.masks import make_identity

dt = mybir.dt
AF = mybir.ActivationFunctionType
OP = mybir.AluOpType

B, N, D = 2, 2048, 512
NCTX = 1024          # cond length
H = 8                # heads
HD = D // H          # 64
EPS = 1e-5
P = 128              # partitions
NCORES = 8
ROWS = 512           # own rows per core
NB = N               # batch rows per core (2048)
SCALE = HD ** -0.5   # 0.125

f32 = dt.float32
bf16 = dt.bfloat16

_CACHED = {}


def _adaln_stats(nc, stat_pool, src_tiles, n_tiles, eps_sb, chunk=4):
    """bn_stats/aggr + rstd/nmr for n_tiles row-tiles. Returns (rstd_all, nmr_all)."""
    mv_all = stat_pool.tile([P, n_tiles, 2], f32)
    rstd_all = stat_pool.tile([P, n_tiles], f32)
    nmr_all = stat_pool.tile([P, n_tiles], f32)
    for c0 in range(0, n_tiles, chunk):
        for it in range(c0, c0 + chunk):
            stats = stat_pool.tile([P, 6], f32, tag="stats")
            nc.vector.bn_stats(stats, src_tiles(it))
            nc.vector.bn_aggr(mv_all[:, it, :], stats)
        cs = slice(c0, c0 + chunk)
        nc.scalar.activation(rstd_all[:, cs], mv_all[:, cs, 1], AF.Sqrt,
                             bias=eps_sb, scale=1.0)
        nc.vector.reciprocal(rstd_all[:, cs], rstd_all[:, cs])
        nc.vector.scalar_tensor_tensor(
            nmr_all[:, cs], mv_all[:, cs, 0], -1.0, rstd_all[:, cs],
            op0=OP.mult, op1=OP.mult,
        )
    return rstd_all, nmr_all


def _adaln_apply(nc, tc, src_tiles, n_tiles, ab, rstd_all, nmr_all, hT,
                 ident_bf16, name, dma_t=False, its=None):
    """xn = (x-mean)*rstd -> transpose -> fused (1+scale)/shift copy.

    dma_t=True routes the transpose through the HWDGE XBAR (SBUF->SBUF DMA)
    instead of the PE array -- frees ~20us of cold-clock PE time in the
    adaln1 window where the PE has nothing else to do anyway.
    """
    with contextlib.ExitStack() as actx:
        xn_pool = actx.enter_context(tc.tile_pool(name=f"{name}_xn", bufs=4))
        if dma_t:
            xnt_pool = actx.enter_context(
                tc.tile_pool(name=f"{name}_xnt", bufs=3))
        else:
            pst_pool = actx.enter_context(
                tc.tile_pool(name=f"{name}_pst", bufs=4, space="PSUM")
            )
        for it in (its if its is not None else range(n_tiles)):
            xn = xn_pool.tile([P, 512], bf16, tag="xn")
            nc.scalar.activation(xn, src_tiles(it), AF.Identity,
                                 bias=nmr_all[:, it:it + 1],
                                 scale=rstd_all[:, it:it + 1])
            if dma_t:
                xnt = xnt_pool.tile([P, 4, P], bf16, tag="xnt")
                for b in range(4):
                    nc.sync.dma_start(xnt[:, b, :], xn[:, b * P:(b + 1) * P],
                                      transpose=True)
            else:
                xnt = pst_pool.tile([P, 4, P], bf16, tag="xnt")
                for b in range(4):
                    nc.tensor.transpose(
                        xnt[:, b, :], xn[:, b * P:(b + 1) * P], ident_bf16
                    )
            for b in range(4):
                nc.vector.tensor_scalar(
                    hT[:, b, it * P:(it + 1) * P], xnt[:, b, :],
                    ab[:, b:b + 1], ab[:, 4 + b:5 + b],
                    op0=OP.mult, op1=OP.add,
                )


def _adaln_to_hT(nc, tc, src_tiles, n_tiles, ab, hT, ident_bf16, eps_sb, name):
    with contextlib.ExitStack() as actx:
        stat_pool = actx.enter_context(tc.tile_pool(name=f"{name}_stat", bufs=4))
        # chunk=2: first xn can issue after only 2 tiles' stats -- this
        # chain is the PE-idle seam between attention phases
        rstd_all, nmr_all = _adaln_stats(nc, stat_pool, src_tiles, n_tiles,
                                         eps_sb, chunk=2)
        _adaln_apply(nc, tc, src_tiles, n_tiles, ab, rstd_all, nmr_all, hT,
                     ident_bf16, name)


VW = HD + 1       # 65: per-head v window [v_h(64) | 1]
VROW = VW * H     # 520: all 8 head windows of one key tile, contiguous


def _vc_tile(nc, act, pool_tag, njt):
    """Flat v tile: njt contiguous 520-col key-tile rows + 63 tail filler.
    Head h's av weight window is cols [jt*520+65h : +128] = [v_h | 1 |
    v_{h+1}[0:63]] -- M=128 without any extra copies (cols past 65 of a
    window produce garbage psum rows that are never read)."""
    vC = act.tile([P, njt * VROW + HD - 1], bf16, tag=pool_tag)
    nc.vector.memset(
        vC[:, 0:njt * VROW].rearrange("p (j h w) -> p j h w", j=njt, h=H)[:, :, :, HD:VW],
        1.0,
    )
    nc.vector.memset(vC[:, njt * VROW:], 0.0)
    return vC


def _vc_copy(nc, vC, jt, ps):
    dst = vC[:, jt * VROW:(jt + 1) * VROW].rearrange("p (h w) -> p h w", h=H)
    nc.vector.tensor_copy(dst[:, :, 0:HD], ps.rearrange("p (h d) -> p h d", h=H))


def _flush_av(nc, vC, njt, pend, avps, pools, ones_row, av_all):
    ht, et, jt = pend
    ps_s, et_pool, dn_pool = pools
    for s in range(2):
        h = 2 * ht + s
        nc.tensor.matmul(
            avps[h], vC[:, jt * VROW + VW * h:jt * VROW + VW * h + P], et[:, s, :],
            start=(jt == 0), stop=(jt == njt - 1),
        )
    if jt == njt - 1:
        # pair finished: issue the whole softmax tail inline so it overlaps
        # the next pair's stream instead of stalling the PE at group end.
        dnms = {}
        for s in range(2):
            dnm_h = dn_pool.tile([1, 512], bf16, tag="dnm")
            nc.vector.tensor_copy(dnm_h, avps[2 * ht + s][HD:HD + 1, :])
            dnms[s] = dnm_h
        # broadcast denom rows across 64 partitions via K=1 matmuls, then
        # one full-width reciprocal for the pair (psum -> sbuf)
        rb = ps_s.tile([P, 2, ROWS], f32, tag="s")
        for s in range(2):
            nc.tensor.matmul(
                rb[s * HD:(s + 1) * HD, 0, :],
                ones_row[0:1, 0:HD],
                dnms[s][0:1, :],
                start=True, stop=True,
            )
        # drain the psum bank with a fast copy so the slow reciprocal runs
        # off the score-pipeline's bank rotation (else next pair stalls 3us)
        rb_f = et_pool.tile([P, ROWS], f32, tag="rbf")
        nc.vector.tensor_copy(rb_f, rb[:, 0, :])
        rb_sb = et_pool.tile([P, ROWS], bf16, tag="rb")
        with nc.allow_low_precision(reason="bf16 softmax recip"):
            nc.vector.reciprocal(rb_sb, rb_f)
        for s in range(2):
            po = 64 * s
            nc.vector.scalar_tensor_tensor(
                av_all[po:po + HD, ht, :],
                avps[2 * ht + s][0:HD, :], 1.0, rb_sb[po:po + HD, :],
                op0=OP.mult, op1=OP.mult,
            )


def _attention(nc, tc, act, qz, kT, vC, njt, wo, ob_row, ones_row,
               x_res, x_out, name):
    """Transposed-score attention for 8 heads over own 512 rows.

    qz: [128, 4 ht, 2, rows]; kT: [128, 4, keys]; vC: [128, njt*520+63].
    All attention matmuls are full 128x128-array (keeps the HAM clock
    gate open). Writes x_out = attn_out @ wo + ob + x_res.
    """
    av_all = act.tile([P, 4, ROWS], bf16, tag="tH")
    with (
        tc.tile_pool(name=f"{name}_ps_s", bufs=2, space="PSUM") as ps_s,
        tc.tile_pool(name=f"{name}_ps_av", bufs=4, space="PSUM") as ps_av,
        tc.tile_pool(name=f"{name}_et", bufs=3) as et_pool,
        tc.tile_pool(name=f"{name}_dn", bufs=4) as dn_pool,
    ):
        pools = (ps_s, et_pool, dn_pool)
        avps = {}
        # software-pipelined: scores/exp for step n+1 issue before the
        # av matmuls of step n, so the in-order PE stream never waits on
        # ScalarE's exp latency. The skew also crosses pair boundaries.
        pend = None   # (ht, et, jt)
        for ht in range(4):
            for s in range(2):
                avp = ps_av.tile([P, ROWS], f32, tag="av")
                avps[2 * ht + s] = avp
            for jt in range(njt):
                sps = ps_s.tile([P, 2, ROWS], f32, tag="s")
                for s in range(2):
                    nc.tensor.matmul(
                        sps[:, s, :],
                        kT[:, ht, jt * P:(jt + 1) * P],
                        qz[:, ht, s, :],
                        start=True, stop=True,
                    )
                et = et_pool.tile([P, 2, ROWS], bf16, tag="et")
                nc.scalar.activation(et, sps, AF.Exp, scale=SCALE)
                if pend is not None:
                    _flush_av(nc, vC, njt, pend, avps, pools, ones_row, av_all)
                pend = (ht, et, jt)
            # flush at pair end is deferred; pend carries over
        if pend is not None:
            _flush_av(nc, vC, njt, pend, avps, pools, ones_row, av_all)
    # out-projection + bias + residual
    with tc.tile_pool(name=f"{name}_ps_o", bufs=2, space="PSUM") as ps_o:
        for it in range(4):
            ps = ps_o.tile([P, D], f32, tag="o")
            for dt_ in range(4):
                nc.tensor.matmul(
                    ps, av_all[:, dt_, it * P:(it + 1) * P], wo[:, dt_, :],
                    start=(dt_ == 0), stop=False,
                )
            nc.tensor.matmul(
                ps, ones_row[0:1, 0:P], ob_row, start=False, stop=True,
            )
            nc.vector.tensor_tensor(x_out[:, it, :], ps, x_res[:, it, :], op=OP.add)


def build(max_phase=5):
    nc = bacc.Bacc(None, target_bir_lowering=False)

    # ---------------- I/O ----------------
    xb = nc.dram_tensor("xb", [NB, D], f32, kind="ExternalInput")
    condb = nc.dram_tensor("condb", [NCTX, D], f32, kind="ExternalInput")
    t_in = nc.dram_tensor("t", [D], f32, kind="ExternalInput")
    nw = {}
    nb_ = {}
    for l in (1, 2, 4):
        nw[l] = nc.dram_tensor(f"n{l}_w", [D, 2 * D], f32, kind="ExternalInput")
        nb_[l] = nc.dram_tensor(f"n{l}_b", [2 * D], f32, kind="ExternalInput")
    aw = {}
    for a in (1, 2):
        for w in "qkvo":
            aw[a, w] = nc.dram_tensor(f"a{a}_{w}", [D, D], f32, kind="ExternalInput")
        aw[a, "ob"] = nc.dram_tensor(f"a{a}_ob", [D], f32, kind="ExternalInput")
    ff_w1 = nc.dram_tensor("ff_w1", [D, 8 * D], f32, kind="ExternalInput")
    ff_b1 = nc.dram_tensor("ff_b1", [8 * D], f32, kind="ExternalInput")
    ff_w2 = nc.dram_tensor("ff_w2", [4 * D, D], f32, kind="ExternalInput")
    ff_b2 = nc.dram_tensor("ff_b2", [D], f32, kind="ExternalInput")
    out = nc.dram_tensor("out", [ROWS, D], f32, kind="ExternalOutput")

    with tile.TileContext(nc) as tc, contextlib.ExitStack() as ctx:
        const = ctx.enter_context(tc.tile_pool(name="const", bufs=1))
        wpool = ctx.enter_context(tc.tile_pool(name="wpool", bufs=1))
        act = ctx.enter_context(tc.tile_pool(name="act", bufs=1))

        ident_bf16 = const.tile([P, P], bf16)
        make_identity(nc, ident_bf16)
        ident_f32 = const.tile([P, P], f32)
        make_identity(nc, ident_f32)
        ones_row = const.tile([1, P], bf16)
        nc.vector.memset(ones_row, 1.0)
        eps_sb = const.tile([P, 1], f32)
        nc.vector.memset(eps_sb, EPS)

        # PE warmup: ~50 dependency-free matmuls fill the otherwise idle
        # startup window and lift the HAM clock gate to 2.4 GHz early
        with tc.tile_pool(name="warm", bufs=1, space="PSUM") as warm_pool:
            wps = warm_pool.tile([P, P], f32)
            for _ in range(50):
                nc.tensor.matmul(wps, ident_bf16, ident_bf16,
                                 start=True, stop=True)

        # t as column tiles [128, 4] bf16 for emb matmul lhsT
        tT = const.tile([P, 4], bf16)
        nc.gpsimd.dma_start(tT, t_in[:].rearrange("(k p) -> p k", p=P))

        # ---------------- norm scale/shift params ----------------
        # emb = t @ nw + nb  -> [1, 1024] -> [128, 8] columns. Only layer 1
        # is needed early; layers 2/4 are deferred past the phase-2 issue so
        # their 2.1MB weight DMAs queue behind the attention weights.
        def _emb(l):
            with (
                tc.tile_pool(name=f"nwp{l}", bufs=1) as nwp,
                tc.tile_pool(name=f"embp{l}", bufs=1) as embp,
                tc.tile_pool(name=f"ps_emb{l}", bufs=2, space="PSUM") as ps_emb,
            ):
                # f32 weights ride the idle HWDGE queues (the SWDGE cast
                # path is saturated with attention/FFN weights); the matmul
                # runs fp32r at full rate since N=512
                nw_sb = nwp.tile([P, 4, 2 * D], bf16, tag="nw")
                nc.gpsimd.dma_start(
                    nw_sb, nw[l][:].rearrange("(k p) n -> p k n", p=P)
                )
                nb_row = embp.tile([1, 2 * D], f32, tag="nbrow")
                nc.sync.dma_start(nb_row, nb_[l][:].rearrange("(a n) -> a n", a=1))
                emb_ps = ps_emb.tile([1, 2 * D], f32, tag="embps")
                for half in range(2):
                    for kt in range(4):
                        nc.tensor.matmul(
                            emb_ps[:, half * D:(half + 1) * D],
                            tT[:, kt:kt + 1],
                            nw_sb[:, kt, half * D:(half + 1) * D],
                            start=(kt == 0), stop=(kt == 3),
                        )
                emb_row = embp.tile([1, 2 * D], f32, tag="embrow")
                nc.vector.tensor_tensor(emb_row, emb_ps, nb_row, op=OP.add)
                # row -> per-partition columns via tiny PE transposes;
                # scale columns (0:4) get the +1 fused into the psum copy
                ab_l = const.tile([P, 8], f32, tag=f"ab{l}")
                for col in range(8):
                    tp = ps_emb.tile([P, 1], f32, tag="embT")
                    nc.tensor.transpose(
                        tp, emb_row[0:1, col * P:(col + 1) * P],
                        ident_f32[0:1, 0:1]
                    )
                    nc.vector.tensor_scalar(
                        ab_l[:, col:col + 1], tp,
                        1.0 if col < 4 else 0.0, None, op0=OP.add,
                    )
            return ab_l

        ab = {1: _emb(1)}

        h1T = act.tile([P, 4, NB], bf16, tag="tA")
        own_x = act.tile([P, 4, D], f32, tag="tE")
        # non-own rows only feed adaln1 -> K/V; bf16 halves their SBUF
        # footprint and doubles DVE stats throughput (DMA casts in flight)
        xrest = act.tile([P, 12, D], bf16, tag="tX")
        x_tiles = {}
        for it in range(16):
            if it < 4:
                dst = own_x[:, it, :]
                nc.sync.dma_start(dst, xb[:][it * P:(it + 1) * P, :])
            else:
                dst = xrest[:, it - 4, :]
                nc.gpsimd.dma_start(dst, xb[:][it * P:(it + 1) * P, :])
            x_tiles[it] = dst

        # adaln1 stats issue first: independent of norm weights, keeps DVE
        # busy while the emb chain waits on its weight DMAs
        n1_stat = ctx.enter_context(tc.tile_pool(name="n1_stat", bufs=4))
        if max_phase >= 1:
            rstd1, nmr1 = _adaln_stats(nc, n1_stat, lambda it: x_tiles[it],
                                       16, eps_sb)




        # ---------------- attention weights (bf16 via DMA cast) ----------
        # a1 stack shares addresses with ff_w1, a2 stack with ff_w2
        # (sequential lifetimes; Tile inserts the WAR deps).
        a_sb = {}
        for a, wtag in ((1, "wbig1"), (2, "wbig2")):
            stack = wpool.tile([P, 4, 4, D], bf16, tag=wtag)
            for wi, w in enumerate("qkvo"):
                nc.gpsimd.dma_start(
                    stack[:, :, wi, :],
                    aw[a, w][:].rearrange("(k p) n -> p k n", p=P),
                )
                a_sb[a, w] = stack[:, :, wi, :]
            ob = wpool.tile([1, D], bf16, tag=f"a{a}ob")
            nc.gpsimd.dma_start(ob, aw[a, "ob"][:].rearrange("(a n) -> a n", a=1))
            a_sb[a, "ob"] = ob


        # ---------------- phase 1: adaln1 apply -> h1T -------------------
        # (for max_phase >= 2 the apply is split inside phase 2 so the
        # own-row projections start after only 4 tiles)
        final = own_x
        if max_phase == 1:
            _adaln_apply(nc, tc, lambda it: x_tiles[it], 16, ab[1], rstd1,
                         nmr1, h1T, ident_bf16, "n1")

        # --- condT: independent of x, fills the idle pre-proj1 PE window ---
        if max_phase >= 4:
            condT = act.tile([P, 4, NCTX], bf16, tag="tE2")
            with (
                tc.tile_pool(name="cin", bufs=6) as cin,
                tc.tile_pool(name="ps_ct", bufs=2, space="PSUM") as ps_ct,
            ):
                for it in range(8):
                    c_sb = cin.tile([P, D], f32, tag="ctile")
                    nc.sync.dma_start(c_sb, condb[:][it * P:(it + 1) * P, :])
                    ct = ps_ct.tile([P, 4, P], f32, tag="ct")
                    for b in range(4):
                        nc.tensor.transpose(
                            ct[:, b, :], c_sb[:, b * P:(b + 1) * P], ident_f32
                        )
                    # ScalarE is idle here; DVE is the backlog (proj copies)
                    for b in range(4):
                        nc.scalar.copy(
                            condT[:, b, it * P:(it + 1) * P], ct[:, b, :]
                        )

        # ---------------- phase 2: projections q1z, k1T, vC1 -------------
        # The adaln1 apply is split: q/k-jc0/v-jt0:4 only read h1T's own-row
        # tiles 0:4, so they issue between the apply halves and the proj
        # pipeline overlaps the remaining 12 apply tiles.
        if max_phase >= 2:
            k1T = act.tile([P, 4, NB], bf16, tag="tB")
            vC1 = _vc_tile(nc, act, "tC", 16)
            q1z = act.tile([P, 4, 2, ROWS], bf16, tag="tD")
            nc.vector.memset(q1z[HD:P, :, 0, :], 0.0)
            nc.vector.memset(q1z[0:HD, :, 1, :], 0.0)

            def _kproj(ps_proj, dt_, jc):
                ps = ps_proj.tile([P, 512], f32, tag="proj")
                for kt in range(4):
                    nc.tensor.matmul(
                        ps,
                        a_sb[1, "k"][:, kt, dt_ * P:(dt_ + 1) * P],
                        h1T[:, kt, jc * 512:(jc + 1) * 512],
                        start=(kt == 0), stop=(kt == 3),
                    )
                nc.vector.tensor_copy(
                    k1T[:, dt_, jc * 512:(jc + 1) * 512], ps
                )

            def _vproj(ps_proj, jt):
                ps = ps_proj.tile([P, 512], f32, tag="proj")
                for kt in range(4):
                    nc.tensor.matmul(
                        ps,
                        h1T[:, kt, jt * P:(jt + 1) * P],
                        a_sb[1, "v"][:, kt, :],
                        start=(kt == 0), stop=(kt == 3),
                    )
                _vc_copy(nc, vC1, jt, ps)

            _adaln_apply(nc, tc, lambda it: x_tiles[it], 16, ab[1], rstd1,
                         nmr1, h1T, ident_bf16, "n1")
            with tc.tile_pool(name="ps_proj1", bufs=4, space="PSUM") as ps_proj:
                for dt_ in range(4):
                    ps = ps_proj.tile([P, 512], f32, tag="proj")
                    for kt in range(4):
                        nc.tensor.matmul(
                            ps,
                            a_sb[1, "q"][:, kt, dt_ * P:(dt_ + 1) * P],
                            h1T[:, kt, 0:ROWS],
                            start=(kt == 0), stop=(kt == 3),
                        )
                    nc.vector.tensor_copy(q1z[0:HD, dt_, 0, :], ps[0:HD, :])
                    nc.vector.tensor_copy(q1z[HD:P, dt_, 1, :], ps[HD:P, :])
                for dt_ in range(4):
                    for jc in range(4):
                        _kproj(ps_proj, dt_, jc)
                    for jt in range(dt_ * 4, dt_ * 4 + 4):
                        _vproj(ps_proj, jt)

        # ------- cross-attn prep: k2T, v2 (condT built pre-proj1) --------
        if max_phase >= 4:
            k2T = act.tile([P, 4, NCTX], bf16, tag="tX")
            vC2 = _vc_tile(nc, act, "tI", 8)
            with tc.tile_pool(name="ps_proj2a", bufs=4, space="PSUM") as ps_proj:
                for dt_ in range(4):
                    for jc in range(2):
                        ps = ps_proj.tile([P, 512], f32, tag="proj")
                        for kt in range(4):
                            nc.tensor.matmul(
                                ps,
                                a_sb[2, "k"][:, kt, dt_ * P:(dt_ + 1) * P],
                                condT[:, kt, jc * 512:(jc + 1) * 512],
                                start=(kt == 0), stop=(kt == 3),
                            )
                        nc.vector.tensor_copy(
                            k2T[:, dt_, jc * 512:(jc + 1) * 512], ps
                        )
                for jt in range(8):
                    ps = ps_proj.tile([P, 512], f32, tag="proj")
                    for kt in range(4):
                        nc.tensor.matmul(
                            ps,
                            condT[:, kt, jt * P:(jt + 1) * P],
                            a_sb[2, "v"][:, kt, :],
                            start=(kt == 0), stop=(kt == 3),
                        )
                    _vc_copy(nc, vC2, jt, ps)

        # deferred adaln2 params: weight DMA queues behind the attention
        # stacks, PE work lands in the proj -> att1 seam
        ab[2] = _emb(2)

        # ---------------- phase 3: attention 1 ---------------------------
        if max_phase >= 3:
            x2 = act.tile([P, 4, D], f32, tag="tF")
            _attention(nc, tc, act, q1z, k1T, vC1, 16, a_sb[1, "o"],
                       a_sb[1, "ob"], ones_row, own_x, x2, "att1")
            final = x2

        # adaln3 params only matter at ~T+300us; the M=1 emb matmuls slot
        # into the att1 -> adaln2 seam where the PE has small gaps anyway
        ab[4] = _emb(4)

        # ---------------- phase 4: adaln2 + cross-attn -------------------
        if max_phase >= 4:
            h2T = act.tile([P, 4, ROWS], bf16, tag="tH")
            _adaln_to_hT(nc, tc, lambda it: x2[:, it, :], 4, ab[2], h2T,
                         ident_bf16, eps_sb, "n2")

            q2z = act.tile([P, 4, 2, ROWS], bf16, tag="tE2")
            nc.vector.memset(q2z[HD:P, :, 0, :], 0.0)
            nc.vector.memset(q2z[0:HD, :, 1, :], 0.0)
            with tc.tile_pool(name="ps_proj2b", bufs=2, space="PSUM") as ps_proj:
                for dt_ in range(4):
                    ps = ps_proj.tile([P, 512], f32, tag="proj")
                    for kt in range(4):
                        nc.tensor.matmul(
                            ps,
                            a_sb[2, "q"][:, kt, dt_ * P:(dt_ + 1) * P],
                            h2T[:, kt, :],
                            start=(kt == 0), stop=(kt == 3),
                        )
                    nc.vector.tensor_copy(q2z[0:HD, dt_, 0, :], ps[0:HD, :])
                    nc.vector.tensor_copy(q2z[HD:P, dt_, 1, :], ps[HD:P, :])

            x3 = act.tile([P, 4, D], f32, tag="tG")
            _attention(nc, tc, act, q2z, k2T, vC2, 8, a_sb[2, "o"],
                       a_sb[2, "ob"], ones_row, x2, x3, "att2")
            final = x3

        # ---------------- phase 5: adaln3 + GEGLU FFN --------------------
        if max_phase >= 5:
            h3T = act.tile([P, 4, ROWS], bf16, tag="tD")
            _adaln_to_hT(nc, tc, lambda it: x3[:, it, :], 4, ab[4], h3T,
                         ident_bf16, eps_sb, "n4")

            # ff_w1 halves live in the dead h1T / vC1 slots so their DMAs
            # start as soon as phase 2 / attention-1 stop reading those,
            # instead of waiting for the a1 weight stack to die.
            w1a = act.tile([P, 4, 4 * D], bf16, tag="tA")
            nc.gpsimd.dma_start(
                w1a, ff_w1[:][:, 0:4 * D].rearrange("(k p) n -> p k n", p=P))
            w1b = act.tile([P, 4, 4 * D], bf16, tag="tC")
            nc.gpsimd.dma_start(
                w1b, ff_w1[:][:, 4 * D:8 * D].rearrange("(k p) n -> p k n", p=P))
            w2_sb = wpool.tile([P, 16, D], bf16, tag="wbig2")
            nc.gpsimd.dma_start(w2_sb, ff_w2[:].rearrange("(k p) n -> p k n", p=P))
            b1_sb = const.tile([P, 32], f32)
            nc.sync.dma_start(b1_sb, ff_b1[:].rearrange("(k p) -> p k", p=P))
            b2_row = const.tile([1, D], bf16)
            nc.gpsimd.dma_start(b2_row, ff_b2[:].rearrange("(a n) -> a n", a=1))

            ugT = act.tile([P, 16, ROWS], bf16, tag="tB")
            with (
                tc.tile_pool(name="ps_z", bufs=4, space="PSUM") as ps_z,
                tc.tile_pool(name="gact", bufs=3) as gact_pool,
            ):
                for ut in range(16):
                    zu = ps_z.tile([P, ROWS], f32, tag="z")
                    zg = ps_z.tile([P, ROWS], f32, tag="z")
                    for kt in range(4):
                        nc.tensor.matmul(
                            zu, w1a[:, kt, ut * P:(ut + 1) * P],
                            h3T[:, kt, :], start=(kt == 0), stop=(kt == 3),
                        )
                    for kt in range(4):
                        nc.tensor.matmul(
                            zg, w1b[:, kt, ut * P:(ut + 1) * P],
                            h3T[:, kt, :], start=(kt == 0), stop=(kt == 3),
                        )
                    gact = gact_pool.tile([P, ROWS], bf16, tag="gact")
                    nc.scalar.activation(
                        gact, zg, AF.Gelu, bias=b1_sb[:, 16 + ut:17 + ut], scale=1.0
                    )
                    nc.vector.scalar_tensor_tensor(
                        ugT[:, ut, :], zu, b1_sb[:, ut:ut + 1], gact,
                        op0=OP.add, op1=OP.mult,
                    )

            out_sb = act.tile([P, 4, D], f32, tag="tC")
            with tc.tile_pool(name="ps_y", bufs=2, space="PSUM") as ps_y:
                for it in range(4):
                    ps = ps_y.tile([P, D], f32, tag="y")
                    for kt in range(16):
                        nc.tensor.matmul(
                            ps, ugT[:, kt, it * P:(it + 1) * P],
                            w2_sb[:, kt, :],
                            start=(kt == 0), stop=False,
                        )
                    nc.tensor.matmul(
                        ps, ones_row[0:1, 0:P], b2_row, start=False, stop=True,
                    )
                    nc.vector.tensor_tensor(
                        out_sb[:, it, :], ps, x3[:, it, :], op=OP.add
                    )
                    # drain each row-block while the next one computes
                    nc.sync.dma_start(out[:][it * P:(it + 1) * P, :],
                                      out_sb[:, it, :])
            final = None

        if final is not None:
            for it_ in range(4):
                nc.sync.dma_start(out[:][it_ * P:(it_ + 1) * P, :],
                                  final[:, it_, :])

    nc.compile()
    return nc


def _shard_inputs(inputs):
    """Build the 8 per-core input maps."""
    x = np.ascontiguousarray(inputs["x"], dtype=np.float32)
    t = np.ascontiguousarray(inputs["t"], dtype=np.float32)
    cond = np.ascontiguousarray(inputs["cond"], dtype=np.float32)
    shared = {}
    for k in ("n1_w", "n1_b", "n2_w", "n2_b", "n4_w", "n4_b",
              "a1_q", "a1_k", "a1_v", "a1_o", "a1_ob",
              "a2_q", "a2_k", "a2_v", "a2_o", "a2_ob",
              "ff_w1", "ff_b1", "ff_w2", "ff_b2"):
        shared[k] = np.ascontiguousarray(inputs[k], dtype=np.float32)
    in_maps = []
    for c in range(NCORES):
        b = c // 4
        r0 = (c % 4) * ROWS
        m = dict(shared)
        m["xb"] = np.ascontiguousarray(np.roll(x[b], -r0, axis=0))
        m["condb"] = np.ascontiguousarray(cond[b])
        m["t"] = np.ascontiguousarray(t[b, 0])
        in_maps.append(m)
    return in_maps


def kernel(**inputs) -> np.ndarray:
    if "nc" not in _CACHED:
        _CACHED["nc"] = build()
    nc = _CACHED["nc"]
    in_maps = _shard_inputs(inputs)
    res = run_bass_kernel_spmd(nc, in_maps, core_ids=list(range(NCORES)))
    outs = [res.results[c]["out"] for c in range(NCORES)]
    full = np.concatenate(outs, axis=0).reshape(B, N, D)
    return full.astype(np.float32)



# revision 30
# speedup vs baseline: 1.1145x; 1.1145x over previous
"""BasicTransformerBlock on 8 TRN2 NeuronCores.

Sharding: sequence-parallel, zero collectives. The [B=2, N=2048, D=512]
residual stream is split into 8 row-blocks of 512 (4 cores per batch
element). Every core recomputes the cheap batch-wide work it needs
(adaln1 + K/V projections over its batch's 2048 rows, cond K/V), and does
attention / FFN only for its own 512 query rows.

Host prepacking: every weight is pre-cast to bf16 and pre-laid-out in its
exact SBUF tile shape ([p, k, n] with d = k*128+p), so every DMA is a
contiguous HWDGE copy -- no SWDGE casts, no on-device rearranges, and cond
arrives already transposed (condT). This halves HBM traffic (~32MB ->
~17MB/core) and removes the cast/transpose work that used to serialize
the first 70us.

Layouts (SBUF tiles are [128 partitions, ...]):
  hT  = normed activations, transposed: [128 p=d%128, 4 dtile, rows] bf16
  kT  = [128 p=dout%128, 4 dtile, rows] bf16   (head pair 2t,2t+1 stacked
        in partitions 0:64 / 64:128 of dtile t)
  qz  = [128, 4 ht, 2 slot, rows] bf16: slot s holds head 2ht+s in its own
        64-partition half, the other half ZERO.
  vC  = [128 p=row%128, njt*520+63] bf16 flat tile: per key tile jt, eight
        contiguous 65-col head windows [v_h(64) | 1]. Head h's av weight
        window is cols [jt*520+65h : +128] = [v_h | 1 | v_{h+1}[0:63]] --
        a full M=128 window with real data, no extra copies. The ones
        column makes the attention-weight row-sum (softmax denominator)
        fall out of the same matmul that computes attn@v; the overrun
        columns produce garbage psum rows 65:127 that are never read.

Every attention matmul is a full 128x128-array op (scores: K=128 via the
stacked head pair against a zero-padded q half; attn@v: M=128 via the
overlapped v windows), which keeps the PE HAM clock gate at 8/8 (2.4GHz).
Both q slots of a head pair share one score matmul (rhs N=1024 bf16).

Scores are computed transposed, sT[j, i], so exp() runs on ScalarE straight
out of PSUM. rstd uses exp(-0.5*ln(var+eps)) so the whole kernel needs only
the natural_log_exp ACT table set until the FFN's gelu -- no table switches
in the attention seams. Softmax denominators are inverted with the
single-op reciprocal_approx_fast instead of the ~3.4us full reciprocal.
"""

import contextlib

import numpy as np
import ml_dtypes

import concourse.bass as bass
import concourse.mybir as mybir
import concourse.tile as tile
from concourse import bacc
from concourse.bass_utils import run_bass_kernel_spmd
from concourse.masks import make_identity

dt = mybir.dt
AF = mybir.ActivationFunctionType
OP = mybir.AluOpType

B, N, D = 2, 2048, 512
NCTX = 1024          # cond length
H = 8                # heads
HD = D // H          # 64
EPS = 1e-5
P = 128              # partitions
NCORES = 8
ROWS = 512           # own rows per core
NB = N               # batch rows per core (2048)
SCALE = HD ** -0.5   # 0.125

f32 = dt.float32
bf16 = dt.bfloat16

_CACHED = {}


def _rstd_from_var(nc, rstd_dst, var_src, lnv_tmp, eps_sb):
    """rstd = exp(-0.5*ln(var+eps)) -- stays inside the natural_log_exp
    ACT table set, so adaln between attention phases never forces a
    sqrt-table switch (which would evict the exp table on the seam)."""
    nc.scalar.activation(lnv_tmp, var_src, AF.Ln, bias=eps_sb, scale=1.0)
    nc.scalar.activation(rstd_dst, lnv_tmp, AF.Exp, scale=-0.5)


def _adaln_stats(nc, stat_pool, src_tiles, n_tiles, eps_sb, chunk=4):
    """bn_stats/aggr + rstd/nmr for n_tiles row-tiles. Returns (rstd_all, nmr_all)."""
    mv_all = stat_pool.tile([P, n_tiles, 2], f32)
    rstd_all = stat_pool.tile([P, n_tiles], f32)
    nmr_all = stat_pool.tile([P, n_tiles], f32)
    for c0 in range(0, n_tiles, chunk):
        for it in range(c0, min(c0 + chunk, n_tiles)):
            stats = stat_pool.tile([P, 6], f32, tag="stats")
            nc.vector.bn_stats(stats, src_tiles(it))
            nc.vector.bn_aggr(mv_all[:, it, :], stats)
        cs = slice(c0, min(c0 + chunk, n_tiles))
        lnv = stat_pool.tile([P, chunk], f32, tag="lnv")
        _rstd_from_var(nc, rstd_all[:, cs], mv_all[:, cs, 1],
                       lnv[:, 0:cs.stop - cs.start], eps_sb)
        nc.vector.scalar_tensor_tensor(
            nmr_all[:, cs], mv_all[:, cs, 0], -1.0, rstd_all[:, cs],
            op0=OP.mult, op1=OP.mult,
        )
    return rstd_all, nmr_all


def _adaln_apply(nc, tc, src_tiles, n_tiles, ab, rstd_all, nmr_all, hT,
                 ident_bf16, name, its=None):
    """xn = (x-mean)*rstd -> transpose -> fused (1+scale)/shift copy.

    Tiles are processed in PAIRS: one [128, 1024] xn activation and
    per-b STTs over 256-wide free dims, halving the per-op overhead that
    used to dominate DVE time in this phase.
    """
    with contextlib.ExitStack() as actx:
        xn_pool = actx.enter_context(tc.tile_pool(name=f"{name}_xn", bufs=3))
        pst_pool = actx.enter_context(
            tc.tile_pool(name=f"{name}_pst", bufs=2, space="PSUM")
        )
        idx = list(its if its is not None else range(n_tiles))
        for i0 in range(0, len(idx), 2):
            pair = idx[i0:i0 + 2]
            xn = xn_pool.tile([P, 2, 512], bf16, tag="xn")
            for j, it in enumerate(pair):
                nc.scalar.activation(xn[:, j, :], src_tiles(it), AF.Identity,
                                     bias=nmr_all[:, it:it + 1],
                                     scale=rstd_all[:, it:it + 1])
            xnt = pst_pool.tile([P, 2, 4, P], bf16, tag="xnt")
            for j, it in enumerate(pair):
                for b in range(4):
                    nc.tensor.transpose(
                        xnt[:, j, b, :], xn[:, j, b * P:(b + 1) * P], ident_bf16
                    )
            if len(pair) == 2 and pair[1] == pair[0] + 1:
                it = pair[0]
                for b in range(4):
                    nc.vector.tensor_scalar(
                        hT[:, b, it * P:(it + 2) * P], xnt[:, :, b, :],
                        ab[:, b:b + 1], ab[:, 4 + b:5 + b],
                        op0=OP.mult, op1=OP.add,
                    )
            else:
                for j, it in enumerate(pair):
                    for b in range(4):
                        nc.vector.tensor_scalar(
                            hT[:, b, it * P:(it + 1) * P], xnt[:, j, b, :],
                            ab[:, b:b + 1], ab[:, 4 + b:5 + b],
                            op0=OP.mult, op1=OP.add,
                        )


def _adaln_to_hT(nc, tc, src_tiles, n_tiles, ab, hT, ident_bf16, eps_sb, name):
    with contextlib.ExitStack() as actx:
        stat_pool = actx.enter_context(tc.tile_pool(name=f"{name}_stat", bufs=4))
        rstd_all, nmr_all = _adaln_stats(nc, stat_pool, src_tiles, n_tiles,
                                         eps_sb, chunk=2)
        _adaln_apply(nc, tc, src_tiles, n_tiles, ab, rstd_all, nmr_all, hT,
                     ident_bf16, name)


VW = HD + 1       # 65: per-head v window [v_h(64) | 1]
VROW = VW * H     # 520: all 8 head windows of one key tile, contiguous


def _vc_memset(nc, vC, njt):
    nc.vector.memset(
        vC[:, 0:njt * VROW].rearrange("p (j h w) -> p j h w", j=njt, h=H)[:, :, :, HD:VW],
        1.0,
    )
    nc.vector.memset(vC[:, njt * VROW:], 0.0)


def _vc_copy(nc, vC, jt, ps):
    """psum [128, 512] (8 heads x 64) -> vC head windows; on ScalarE to keep
    DVE free for the adaln applies that share this phase."""
    dst = vC[:, jt * VROW:(jt + 1) * VROW].rearrange("p (h w) -> p h w", h=H)
    nc.scalar.copy(dst[:, :, 0:HD], ps.rearrange("p (h d) -> p h d", h=H))


def _flush_av(nc, vC, njt, pend, avps, pools, ones_row, av_all):
    ht, et, jt = pend
    ps_s, et_pool, dn_pool = pools
    avp = avps[ht]
    for s in range(2):
        h = 2 * ht + s
        nc.tensor.matmul(
            avp[:, s, :], vC[:, jt * VROW + VW * h:jt * VROW + VW * h + P],
            et[:, s, :],
            start=(jt == 0), stop=(jt == njt - 1),
        )
    if jt == njt - 1:
        # pair finished: softmax tail. Both slots' denominators sit in psum
        # row 64 of the pair tile; copy to SBUF bf16, K=1 broadcast matmuls
        # of the RAW denominators into a [128, 512] psum, drain to f32, one
        # [128,512] fast approx reciprocal (512 elems/lane), then scale.
        dnm = dn_pool.tile([1, 2, ROWS], bf16, tag="dnm")
        nc.vector.tensor_copy(dnm, avp[HD:HD + 1, :, :])
        rb = ps_s.tile([P, 2, ROWS], f32, tag="s")
        for s in range(2):
            nc.tensor.matmul(
                rb[s * HD:(s + 1) * HD, 0, :],
                ones_row[0:1, 0:HD],
                dnm[0:1, s, :],
                start=True, stop=True,
            )
        rb_f = dn_pool.tile([P, ROWS], f32, tag="rbf")
        nc.vector.tensor_copy(rb_f, rb[:, 0, :])
        rb_r = dn_pool.tile([P, ROWS], f32, tag="rbr")
        nc.vector.reciprocal_approx_fast(rb_r, rb_f)
        for s in range(2):
            po = 64 * s
            nc.vector.scalar_tensor_tensor(
                av_all[po:po + HD, ht, :],
                avp[0:HD, s, :], 1.0, rb_r[po:po + HD, :],
                op0=OP.mult, op1=OP.mult,
            )


def _attention(nc, tc, act, qz, kT, vC, njt, wo, ob_row, ones_row,
               x_res, x_out, name):
    """Transposed-score attention for 8 heads over own 512 rows.

    qz: [128, 4 ht, 2, rows]; kT: [128, 4, keys]; vC: [128, njt*520+63].
    All attention matmuls are full 128x128-array (keeps the HAM clock
    gate open). Writes x_out = attn_out @ wo + ob + x_res.
    """
    av_all = act.tile([P, 4, ROWS], bf16, tag="tH")
    with (
        tc.tile_pool(name=f"{name}_ps_s", bufs=2, space="PSUM") as ps_s,
        tc.tile_pool(name=f"{name}_ps_av", bufs=2, space="PSUM") as ps_av,
        tc.tile_pool(name=f"{name}_et", bufs=3) as et_pool,
        tc.tile_pool(name=f"{name}_dn", bufs=1) as dn_pool,
    ):
        pools = (ps_s, et_pool, dn_pool)
        avps = {}
        # software-pipelined: scores/exp for step n+1 issue before the
        # av matmuls of step n, so the in-order PE stream never waits on
        # ScalarE's exp latency. The skew also crosses pair boundaries.
        pend = None   # (ht, et, jt)
        for ht in range(4):
            avps[ht] = ps_av.tile([P, 2, ROWS], f32, tag="av", name="avp")
            for jt in range(njt):
                sps = ps_s.tile([P, 2, ROWS], f32, tag="s")
                for s in range(2):
                    nc.tensor.matmul(
                        sps[:, s, :],
                        kT[:, ht, jt * P:(jt + 1) * P],
                        qz[:, ht, s, :],
                        start=True, stop=True,
                    )
                et = et_pool.tile([P, 2, ROWS], bf16, tag="et")
                nc.scalar.activation(et, sps, AF.Exp, scale=SCALE)
                if pend is not None:
                    _flush_av(nc, vC, njt, pend, avps, pools, ones_row, av_all)
                pend = (ht, et, jt)
            # flush at pair end is deferred; pend carries over
        if pend is not None:
            _flush_av(nc, vC, njt, pend, avps, pools, ones_row, av_all)
    # out-projection + bias + residual
    with tc.tile_pool(name=f"{name}_ps_o", bufs=2, space="PSUM") as ps_o:
        for it in range(4):
            ps = ps_o.tile([P, D], f32, tag="o")
            for dt_ in range(4):
                nc.tensor.matmul(
                    ps, av_all[:, dt_, it * P:(it + 1) * P], wo[:, dt_, :],
                    start=(dt_ == 0), stop=False,
                )
            nc.tensor.matmul(
                ps, ones_row[0:1, 0:P], ob_row, start=False, stop=True,
            )
            nc.vector.tensor_tensor(x_out[:, it, :], ps, x_res[:, it, :], op=OP.add)
    return av_all


def build(max_phase=5, debug=False):
    nc = bacc.Bacc(None, target_bir_lowering=False)

    # ---------------- I/O (host-prepacked layouts) ----------------
    xo_d = nc.dram_tensor("xo", [P, 4, D], f32, kind="ExternalInput")
    xr_d = nc.dram_tensor("xr", [P, 12, D], bf16, kind="ExternalInput")
    condT_d = nc.dram_tensor("condT", [P, 4, NCTX], bf16, kind="ExternalInput")
    tT_d = nc.dram_tensor("tT", [P, 4], bf16, kind="ExternalInput")
    nw = {}
    nb_ = {}
    for l in (1, 2, 4):
        nw[l] = nc.dram_tensor(f"n{l}_w", [P, 4, 2 * D], bf16, kind="ExternalInput")
        nb_[l] = nc.dram_tensor(f"n{l}_b", [P, 8], f32, kind="ExternalInput")
    aw = {}
    for a in (1, 2):
        aw[a] = nc.dram_tensor(f"a{a}_w", [P, 4, 4, D], bf16, kind="ExternalInput")
        aw[a, "ob"] = nc.dram_tensor(f"a{a}_ob", [1, D], bf16, kind="ExternalInput")
    w1a_d = nc.dram_tensor("ff_w1a", [P, 4, 4 * D], bf16, kind="ExternalInput")
    w1b_d = nc.dram_tensor("ff_w1b", [P, 4, 4 * D], bf16, kind="ExternalInput")
    w2_d = nc.dram_tensor("ff_w2", [P, 16, D], bf16, kind="ExternalInput")
    b1_d = nc.dram_tensor("ff_b1", [P, 32], f32, kind="ExternalInput")
    b2_d = nc.dram_tensor("ff_b2", [1, D], bf16, kind="ExternalInput")
    out = nc.dram_tensor("out", [ROWS, D], f32, kind="ExternalOutput")

    with tile.TileContext(nc) as tc, contextlib.ExitStack() as ctx:
        const = ctx.enter_context(tc.tile_pool(name="const", bufs=1))
        wpool = ctx.enter_context(tc.tile_pool(name="wpool", bufs=1))
        act = ctx.enter_context(tc.tile_pool(name="act", bufs=1))

        ident_bf16 = const.tile([P, P], bf16)
        make_identity(nc, ident_bf16)
        ident_f32 = const.tile([P, P], f32)
        make_identity(nc, ident_f32)
        ones_row = const.tile([1, P], bf16)
        nc.vector.memset(ones_row, 1.0)
        eps_sb = const.tile([P, 1], f32)
        nc.vector.memset(eps_sb, EPS)

        # PE warmup: dependency-free matmuls fill the otherwise idle
        # startup window and lift the HAM clock gate to 2.4 GHz early
        with tc.tile_pool(name="warm", bufs=1, space="PSUM") as warm_pool:
            wps = warm_pool.tile([P, P], f32)
            for _ in range(50):
                nc.tensor.matmul(wps, ident_bf16, ident_bf16,
                                 start=True, stop=True)

        # ------- input DMAs, all contiguous HWDGE, in priority order -----
        tT = const.tile([P, 4], bf16)
        nc.sync.dma_start(tT, tT_d[:])
        nw1_sb = wpool.tile([P, 4, 2 * D], bf16, tag="nw1")
        nc.sync.dma_start(nw1_sb, nw[1][:])
        nb1_col = const.tile([P, 8], f32, tag="nb1")
        nc.sync.dma_start(nb1_col, nb_[1][:])

        own_x = act.tile([P, 4, D], f32, tag="tE")
        x_tiles = {}
        for it in range(4):
            nc.sync.dma_start(own_x[:, it, :], xo_d[:][:, it, :])
            x_tiles[it] = own_x[:, it, :]
        a_sb = {}
        stack1 = wpool.tile([P, 4, 4, D], bf16, tag="wbig1")
        nc.sync.dma_start(stack1, aw[1][:])
        for wi, w in enumerate("qkvo"):
            a_sb[1, w] = stack1[:, :, wi, :]
        xrest = act.tile([P, 12, D], bf16, tag="tX")
        for c in range(3):
            nc.sync.dma_start(xrest[:, c * 4:(c + 1) * 4, :],
                              xr_d[:][:, c * 4:(c + 1) * 4, :])
            for it in range(4):
                x_tiles[4 + c * 4 + it] = xrest[:, c * 4 + it, :]
        condT = act.tile([P, 4, NCTX], bf16, tag="tE2")
        nc.sync.dma_start(condT, condT_d[:])
        stack2 = wpool.tile([P, 4, 4, D], bf16, tag="wbig2")
        nc.sync.dma_start(stack2, aw[2][:])
        for wi, w in enumerate("qkvo"):
            a_sb[2, w] = stack2[:, :, wi, :]
        for a in (1, 2):
            ob = wpool.tile([1, D], bf16, tag=f"a{a}ob")
            nc.sync.dma_start(ob, aw[a, "ob"][:])
            a_sb[a, "ob"] = ob

        # norm weights for layers 2/4 + FFN weights ride the gpsimd queue
        # so the sync queue's critical-path loads are never behind them.
        nw_sb = {1: nw1_sb}
        nb_col = {1: nb1_col}
        for l in (2, 4):
            nw_sb[l] = wpool.tile([P, 4, 2 * D], bf16, tag=f"nw{l}",
                                  name=f"nw{l}_sb")
            nc.gpsimd.dma_start(nw_sb[l], nw[l][:])
            nb_col[l] = const.tile([P, 8], f32, tag=f"nb{l}",
                                   name=f"nb{l}_col")
            nc.gpsimd.dma_start(nb_col[l], nb_[l][:])

        # ---------------- norm scale/shift params ----------------
        def _emb(l):
            with (
                tc.tile_pool(name=f"embp{l}", bufs=1) as embp,
                tc.tile_pool(name=f"ps_emb{l}", bufs=2, space="PSUM") as ps_emb,
            ):
                emb_ps = ps_emb.tile([1, 2 * D], f32, tag="embps")
                for half in range(2):
                    for kt in range(4):
                        nc.tensor.matmul(
                            emb_ps[:, half * D:(half + 1) * D],
                            tT[:, kt:kt + 1],
                            nw_sb[l][:, kt, half * D:(half + 1) * D],
                            start=(kt == 0), stop=(kt == 3),
                        )
                emb_row = embp.tile([1, 2 * D], f32, tag="embrow")
                nc.vector.tensor_copy(emb_row, emb_ps)
                # row -> per-partition columns via tiny PE transposes;
                # bias column add + the +1 for scale cols fused into the
                # psum drain
                ab_l = const.tile([P, 8], f32, tag=f"ab{l}")
                for col in range(8):
                    tp = ps_emb.tile([P, 1], f32, tag="embT")
                    nc.tensor.transpose(
                        tp, emb_row[0:1, col * P:(col + 1) * P],
                        ident_f32[0:1, 0:1]
                    )
                    nc.vector.scalar_tensor_tensor(
                        ab_l[:, col:col + 1], tp,
                        1.0 if col < 4 else 0.0, nb_col[l][:, col:col + 1],
                        op0=OP.add, op1=OP.add,
                    )
            return ab_l

        ab = {1: _emb(1)}

        h1T = act.tile([P, 4, NB], bf16, tag="tA")

        # adaln1 stats issue first: independent of norm weights, keeps DVE
        # busy while the emb chain waits on its weight DMAs
        n1_stat = ctx.enter_context(tc.tile_pool(name="n1_stat", bufs=4))
        if max_phase >= 1:
            rstd1, nmr1 = _adaln_stats(nc, n1_stat, lambda it: x_tiles[it],
                                       16, eps_sb)

        # ---------------- phase 2: projections q1z, k1T, vC1 -------------
        # Apply tiles 0-3 (own rows) first so the q projection starts as
        # early as possible; k/v follow as the remaining applies complete.
        if max_phase >= 2:
            k1T = act.tile([P, 4, NB], bf16, tag="tB")
            vC1 = act.tile([P, 16 * VROW + HD - 1], bf16, tag="tC")
            _vc_memset(nc, vC1, 16)
            q1z = act.tile([P, 4, 2, ROWS], bf16, tag="tD")
            nc.vector.memset(q1z[HD:P, :, 0, :], 0.0)
            nc.vector.memset(q1z[0:HD, :, 1, :], 0.0)

            def _kproj(ps_k, kT_dst, h_src, wkey, dt_, jc):
                # two N=512 matmuls cover 1024 keys into one [P,2,512]
                # psum tile; a single copy drains both halves
                ps = ps_k.tile([P, 2, 512], f32, tag="projk")
                for half in range(2):
                    for kt in range(4):
                        nc.tensor.matmul(
                            ps[:, half, :],
                            a_sb[wkey][:, kt, dt_ * P:(dt_ + 1) * P],
                            h_src[:, kt, jc * 1024 + half * 512:
                                  jc * 1024 + (half + 1) * 512],
                            start=(kt == 0), stop=(kt == 3),
                        )
                nc.vector.tensor_copy(
                    kT_dst[:, dt_, jc * 1024:(jc + 1) * 1024].rearrange(
                        "p (a b) -> p a b", a=2),
                    ps,
                )

            def _vproj(ps_qv, vC_dst, h_src, wkey, jt):
                ps = ps_qv.tile([P, 512], f32, tag="proj")
                for kt in range(4):
                    nc.tensor.matmul(
                        ps,
                        h_src[:, kt, jt * P:(jt + 1) * P],
                        a_sb[wkey][:, kt, :],
                        start=(kt == 0), stop=(kt == 3),
                    )
                _vc_copy(nc, vC_dst, jt, ps)

            def _qproj(ps_qv, qz_dst, hT_src, wkey):
                for dt_ in range(4):
                    ps = ps_qv.tile([P, 512], f32, tag="proj")
                    for kt in range(4):
                        nc.tensor.matmul(
                            ps,
                            a_sb[wkey][:, kt, dt_ * P:(dt_ + 1) * P],
                            hT_src[:, kt, 0:ROWS],
                            start=(kt == 0), stop=(kt == 3),
                        )
                    nc.vector.tensor_copy(qz_dst[0:HD, dt_, 0, :], ps[0:HD, :])
                    nc.vector.tensor_copy(qz_dst[HD:P, dt_, 1, :], ps[HD:P, :])

            _adaln_apply(nc, tc, lambda it: x_tiles[it], 16, ab[1], rstd1,
                         nmr1, h1T, ident_bf16, "n1a", its=range(0, 4))
            with (
                tc.tile_pool(name="ps_k1", bufs=2, space="PSUM") as ps_k,
                tc.tile_pool(name="ps_qv1", bufs=2, space="PSUM") as ps_qv,
            ):
                _qproj(ps_qv, q1z, h1T, (1, "q"))
                _adaln_apply(nc, tc, lambda it: x_tiles[it], 16, ab[1], rstd1,
                             nmr1, h1T, ident_bf16, "n1b", its=range(4, 8))
                for dt_ in range(4):
                    _kproj(ps_k, k1T, h1T, (1, "k"), dt_, 0)
                for jt in range(0, 8):
                    _vproj(ps_qv, vC1, h1T, (1, "v"), jt)
                _adaln_apply(nc, tc, lambda it: x_tiles[it], 16, ab[1], rstd1,
                             nmr1, h1T, ident_bf16, "n1c", its=range(8, 16))
                for dt_ in range(4):
                    _kproj(ps_k, k1T, h1T, (1, "k"), dt_, 1)
                for jt in range(8, 16):
                    _vproj(ps_qv, vC1, h1T, (1, "v"), jt)

        if debug and max_phase >= 2:
            dbg_h1 = nc.dram_tensor("dbg_h1", [P, 4, NB], bf16, kind="ExternalOutput")
            nc.sync.dma_start(dbg_h1[:], h1T)
            dbg_k1 = nc.dram_tensor("dbg_k1", [P, 4, NB], bf16, kind="ExternalOutput")
            nc.sync.dma_start(dbg_k1[:], k1T)
            dbg_q1 = nc.dram_tensor("dbg_q1", [P, 4, 2, ROWS], bf16,
                                    kind="ExternalOutput")
            nc.sync.dma_start(dbg_q1[:], q1z)
            dbg_v1 = nc.dram_tensor("dbg_v1", [P, 16 * VROW + HD - 1], bf16,
                                    kind="ExternalOutput")
            nc.sync.dma_start(dbg_v1[:], vC1)

        # deferred adaln2 params: PE work lands in the proj -> att1 seam
        ab[2] = _emb(2)

        # ---------------- phase 3: attention 1 ---------------------------
        if max_phase >= 3:
            x2 = act.tile([P, 4, D], f32, tag="tF")
            av_dbg = _attention(nc, tc, act, q1z, k1T, vC1, 16, a_sb[1, "o"],
                                a_sb[1, "ob"], ones_row, own_x, x2, "att1")
            if debug:
                dbg_av = nc.dram_tensor("dbg_av", [P, 4, ROWS], bf16,
                                        kind="ExternalOutput")
                nc.sync.dma_start(dbg_av[:], av_dbg)
            final = x2
        else:
            final = own_x

        # FFN weights: issued here so their DMAs run during attention-1.
        # w1 halves live in the dead h1T / vC1 slots (WAR deps handle it).
        if max_phase >= 5:
            w1a = act.tile([P, 4, 4 * D], bf16, tag="tA")
            nc.gpsimd.dma_start(w1a, w1a_d[:])
            w1b = act.tile([P, 4, 4 * D], bf16, tag="tC")
            nc.gpsimd.dma_start(w1b, w1b_d[:])
            w2_sb = wpool.tile([P, 16, D], bf16, tag="wff2")
            nc.gpsimd.dma_start(w2_sb, w2_d[:])
            b1_sb = const.tile([P, 32], f32)
            nc.gpsimd.dma_start(b1_sb, b1_d[:])
            b2_row = const.tile([1, D], bf16)
            nc.gpsimd.dma_start(b2_row, b2_d[:])

        ab[4] = _emb(4)

        # ------- seam: adaln2 (DVE/Scalar) runs while the PE does the ----
        # ------- cross-attn K/V projections (independent of x2) ----------
        if max_phase >= 4:
            h2T = act.tile([P, 4, ROWS], bf16, tag="tH")
            with contextlib.ExitStack() as sctx:
                stat2 = sctx.enter_context(tc.tile_pool(name="n2_stat", bufs=4))
                rstd2, nmr2 = _adaln_stats(nc, stat2, lambda it: x2[:, it, :],
                                           4, eps_sb, chunk=2)

                k2T = act.tile([P, 4, NCTX], bf16, tag="tX")
                vC2 = act.tile([P, 8 * VROW + HD - 1], bf16, tag="tI")
                _vc_memset(nc, vC2, 8)
                q2z = act.tile([P, 4, 2, ROWS], bf16, tag="tD")
                nc.vector.memset(q2z[HD:P, :, 0, :], 0.0)
                nc.vector.memset(q2z[0:HD, :, 1, :], 0.0)
                with (
                    tc.tile_pool(name="ps_k2", bufs=2, space="PSUM") as ps_k,
                    tc.tile_pool(name="ps_qv2", bufs=2, space="PSUM") as ps_qv,
                ):
                    for dt_ in range(4):
                        _kproj(ps_k, k2T, condT, (2, "k"), dt_, 0)
                    for jt in range(8):
                        _vproj(ps_qv, vC2, condT, (2, "v"), jt)
                    _adaln_apply(nc, tc, lambda it: x2[:, it, :], 4, ab[2],
                                 rstd2, nmr2, h2T, ident_bf16, "n2")
                    _qproj(ps_qv, q2z, h2T, (2, "q"))

            x3 = act.tile([P, 4, D], f32, tag="tG")
            _attention(nc, tc, act, q2z, k2T, vC2, 8, a_sb[2, "o"],
                       a_sb[2, "ob"], ones_row, x2, x3, "att2")
            final = x3

        # ---------------- phase 5: adaln3 + GEGLU FFN --------------------
        if max_phase >= 5:
            h3T = act.tile([P, 4, ROWS], bf16, tag="tD")
            _adaln_to_hT(nc, tc, lambda it: x3[:, it, :], 4, ab[4], h3T,
                         ident_bf16, eps_sb, "n4")

            ugT = act.tile([P, 16, ROWS], bf16, tag="tB")
            with (
                tc.tile_pool(name="ps_z", bufs=4, space="PSUM") as ps_z,
                tc.tile_pool(name="gact", bufs=3) as gact_pool,
            ):
                for ut in range(16):
                    zu = ps_z.tile([P, ROWS], f32, tag="z")
                    zg = ps_z.tile([P, ROWS], f32, tag="z")
                    # zg first: gelu (ScalarE) starts 4 matmuls earlier,
                    # overlapping the zu matmuls
                    for kt in range(4):
                        nc.tensor.matmul(
                            zg, w1b[:, kt, ut * P:(ut + 1) * P],
                            h3T[:, kt, :], start=(kt == 0), stop=(kt == 3),
                        )
                    for kt in range(4):
                        nc.tensor.matmul(
                            zu, w1a[:, kt, ut * P:(ut + 1) * P],
                            h3T[:, kt, :], start=(kt == 0), stop=(kt == 3),
                        )
                    gact = gact_pool.tile([P, ROWS], bf16, tag="gact")
                    nc.scalar.activation(
                        gact, zg, AF.Gelu, bias=b1_sb[:, 16 + ut:17 + ut], scale=1.0
                    )
                    nc.vector.scalar_tensor_tensor(
                        ugT[:, ut, :], zu, b1_sb[:, ut:ut + 1], gact,
                        op0=OP.add, op1=OP.mult,
                    )

            out_sb = act.tile([P, 4, D], f32, tag="tC")
            with tc.tile_pool(name="ps_y", bufs=2, space="PSUM") as ps_y:
                for it in range(4):
                    ps = ps_y.tile([P, D], f32, tag="y")
                    for kt in range(16):
                        nc.tensor.matmul(
                            ps, ugT[:, kt, it * P:(it + 1) * P],
                            w2_sb[:, kt, :],
                            start=(kt == 0), stop=False,
                        )
                    nc.tensor.matmul(
                        ps, ones_row[0:1, 0:P], b2_row, start=False, stop=True,
                    )
                    nc.vector.tensor_tensor(
                        out_sb[:, it, :], ps, x3[:, it, :], op=OP.add
                    )
                    # drain each row-block while the next one computes
                    nc.sync.dma_start(out[:][it * P:(it + 1) * P, :],
                                      out_sb[:, it, :])
            final = None

        if final is not None:
            for it_ in range(4):
                nc.sync.dma_start(out[:][it_ * P:(it_ + 1) * P, :],
                                  final[:, it_, :])

    nc.compile()
    return nc


def _pack_w(w):
    """[K, N] f32 -> [128, K//128, N] bf16 (d = k*128 + p)."""
    K, Nc = w.shape
    return np.ascontiguousarray(
        w.reshape(K // P, P, Nc).transpose(1, 0, 2).astype(ml_dtypes.bfloat16))


def _shard_inputs(inputs):
    """Build the 8 per-core input maps (prepacked SBUF layouts, bf16)."""
    bf = ml_dtypes.bfloat16
    x = np.ascontiguousarray(inputs["x"], dtype=np.float32)
    t = np.ascontiguousarray(inputs["t"], dtype=np.float32)
    cond = np.ascontiguousarray(inputs["cond"], dtype=np.float32)

    shared = {}
    for l in (1, 2, 4):
        shared[f"n{l}_w"] = _pack_w(inputs[f"n{l}_w"].astype(np.float32))
        shared[f"n{l}_b"] = np.ascontiguousarray(
            inputs[f"n{l}_b"].astype(np.float32).reshape(8, P).T)
    for a in (1, 2):
        stack = np.stack(
            [_pack_w(inputs[f"a{a}_{w}"].astype(np.float32)) for w in "qkvo"],
            axis=2)  # [128, 4, 4, 512]
        shared[f"a{a}_w"] = np.ascontiguousarray(stack)
        shared[f"a{a}_ob"] = np.ascontiguousarray(
            inputs[f"a{a}_ob"].astype(np.float32).reshape(1, D).astype(bf))
    ff_w1 = inputs["ff_w1"].astype(np.float32)
    shared["ff_w1a"] = _pack_w(ff_w1[:, 0:4 * D])
    shared["ff_w1b"] = _pack_w(ff_w1[:, 4 * D:8 * D])
    shared["ff_w2"] = _pack_w(inputs["ff_w2"].astype(np.float32))
    shared["ff_b1"] = np.ascontiguousarray(
        inputs["ff_b1"].astype(np.float32).reshape(32, P).T)
    shared["ff_b2"] = np.ascontiguousarray(
        inputs["ff_b2"].astype(np.float32).reshape(1, D).astype(bf))

    in_maps = []
    for c in range(NCORES):
        b = c // 4
        r0 = (c % 4) * ROWS
        m = dict(shared)
        xr_full = np.roll(x[b], -r0, axis=0)          # [2048, 512]
        m["xo"] = np.ascontiguousarray(
            xr_full[0:ROWS].reshape(4, P, D).transpose(1, 0, 2))
        m["xr"] = np.ascontiguousarray(
            xr_full[ROWS:].reshape(12, P, D).transpose(1, 0, 2).astype(bf))
        # condT[p, kt, j] = cond[b, j, kt*128+p]
        m["condT"] = np.ascontiguousarray(
            cond[b].reshape(NCTX, 4, P).transpose(2, 1, 0).astype(bf))
        m["tT"] = np.ascontiguousarray(
            t[b, 0].reshape(4, P).T.astype(bf))
        in_maps.append(m)
    return in_maps


def kernel(**inputs) -> np.ndarray:
    if "nc" not in _CACHED:
        _CACHED["nc"] = build()
    nc = _CACHED["nc"]
    in_maps = _shard_inputs(inputs)
    res = run_bass_kernel_spmd(nc, in_maps, core_ids=list(range(NCORES)))
    outs = [res.results[c]["out"] for c in range(NCORES)]
    full = np.concatenate(outs, axis=0).reshape(B, N, D)
    return full.astype(np.float32)


# revision 37
# speedup vs baseline: 1.1420x; 1.0247x over previous
"""BasicTransformerBlock on 8 TRN2 NeuronCores.

Sharding: sequence-parallel, zero collectives. The [B=2, N=2048, D=512]
residual stream is split into 8 row-blocks of 512 (4 cores per batch
element). Every core recomputes the cheap batch-wide work it needs
(adaln1 + K/V projections over its batch's 2048 rows, cond K/V), and does
attention / FFN only for its own 512 query rows.

Host prepacking: every weight is pre-cast to bf16 and pre-laid-out in its
exact SBUF tile shape ([p, k, n] with d = k*128+p), so every DMA is a
contiguous HWDGE copy -- no SWDGE casts, no on-device rearranges, and cond
arrives already transposed (condT). This halves HBM traffic (~32MB ->
~17MB/core) and removes the cast/transpose work that used to serialize
the first 70us.

Layouts (SBUF tiles are [128 partitions, ...]):
  hT  = normed activations, transposed: [128 p=d%128, 4 dtile, rows] bf16
  kT  = [128 p=dout%128, 4 dtile, rows] bf16   (head pair 2t,2t+1 stacked
        in partitions 0:64 / 64:128 of dtile t)
  qz  = [128, 4 ht, 2 slot, rows] bf16: slot s holds head 2ht+s in its own
        64-partition half, the other half ZERO.
  vC  = [128 p=row%128, njt*520+63] bf16 flat tile: per key tile jt, eight
        contiguous 65-col head windows [v_h(64) | 1]. Head h's av weight
        window is cols [jt*520+65h : +128] = [v_h | 1 | v_{h+1}[0:63]] --
        a full M=128 window with real data, no extra copies. The ones
        column makes the attention-weight row-sum (softmax denominator)
        fall out of the same matmul that computes attn@v; the overrun
        columns produce garbage psum rows 65:127 that are never read.

Every attention matmul is a full 128x128-array op (scores: K=128 via the
stacked head pair against a zero-padded q half; attn@v: M=128 via the
overlapped v windows), which keeps the PE HAM clock gate at 8/8 (2.4GHz).
Both q slots of a head pair share one score matmul (rhs N=1024 bf16).

Scores are computed transposed, sT[j, i], so exp() runs on ScalarE straight
out of PSUM. rstd uses exp(-0.5*ln(var+eps)) so the whole kernel needs only
the natural_log_exp ACT table set until the FFN's gelu -- no table switches
in the attention seams. Softmax denominators are inverted with the
single-op reciprocal_approx_fast instead of the ~3.4us full reciprocal.
"""

import contextlib

import numpy as np
import ml_dtypes

import concourse.bass as bass
import concourse.mybir as mybir
import concourse.tile as tile
from concourse import bacc
from concourse.bass_utils import run_bass_kernel_spmd
from concourse.masks import make_identity

dt = mybir.dt
AF = mybir.ActivationFunctionType
OP = mybir.AluOpType

B, N, D = 2, 2048, 512
NCTX = 1024          # cond length
H = 8                # heads
HD = D // H          # 64
EPS = 1e-5
P = 128              # partitions
NCORES = 8
ROWS = 512           # own rows per core
NB = N               # batch rows per core (2048)
SCALE = HD ** -0.5   # 0.125

f32 = dt.float32
bf16 = dt.bfloat16

_CACHED = {}


i32 = dt.int32
RSQRT_MAGIC = 0x5F3759DF


def _rstd_dve(nc, stat_pool, rstd_dst, var_src, n):
    """rstd = 1/sqrt(var+eps) entirely on DVE: magic-constant seed + two
    Newton-Raphson passes (rel err ~4e-6). Keeps ScalarE's ACT table free
    for exp/gelu -- no sqrt-set loads between the attention phases."""
    ve = stat_pool.tile([P, n], f32, tag="ve", name="ve")
    nc.vector.tensor_scalar(ve, var_src, EPS, None, op0=OP.add)
    y = stat_pool.tile([P, n], f32, tag="rsy", name="rsy")
    # seed bits: NOT((i>>1) - MAGIC) == MAGIC - (i>>1) - 1. int32 domain:
    # every intermediate stays in range (uint32 subtract would saturate).
    nc.vector.tensor_scalar(y.bitcast(i32), ve.bitcast(i32), 1, None,
                            op0=OP.logical_shift_right)
    nc.vector.tensor_scalar(y.bitcast(i32), y.bitcast(i32), RSQRT_MAGIC, None,
                            op0=OP.subtract)
    nc.vector.tensor_scalar(y.bitcast(i32), y.bitcast(i32), -1, None,
                            op0=OP.bitwise_xor)
    t = stat_pool.tile([P, n], f32, tag="rst", name="rst")
    for _ in range(2):
        nc.vector.tensor_tensor(t, ve, y, op=OP.mult)
        nc.vector.tensor_tensor(t, t, y, op=OP.mult)
        nc.vector.tensor_scalar(t, t, -0.5, 1.5, op0=OP.mult, op1=OP.add)
        nc.vector.tensor_tensor(y, y, t, op=OP.mult)
    nc.vector.tensor_copy(rstd_dst, y)


def _adaln_stats(nc, stat_pool, src_tiles, n_tiles, eps_sb, groups=None):
    """bn_stats/aggr + rstd/nmr for n_tiles row-tiles. `groups` is a list of
    (start, stop) tile ranges; rstd/nmr for a group issue as soon as that
    group's stats are done. Returns (rstd_all, nmr_all)."""
    if groups is None:
        groups = [(0, n_tiles)]
    mv_all = stat_pool.tile([P, n_tiles, 2], f32)
    rstd_all = stat_pool.tile([P, n_tiles], f32)
    nmr_all = stat_pool.tile([P, n_tiles], f32)
    for g0, g1 in groups:
        for it in range(g0, g1):
            stats = stat_pool.tile([P, 6], f32, tag="stats")
            nc.vector.bn_stats(stats, src_tiles(it))
            nc.vector.bn_aggr(mv_all[:, it, :], stats)
        cs = slice(g0, g1)
        _rstd_dve(nc, stat_pool, rstd_all[:, cs], mv_all[:, cs, 1], g1 - g0)
        nc.vector.scalar_tensor_tensor(
            nmr_all[:, cs], mv_all[:, cs, 0], -1.0, rstd_all[:, cs],
            op0=OP.mult, op1=OP.mult,
        )
    return rstd_all, nmr_all


def _adaln_apply(nc, tc, src_tiles, n_tiles, ab, rstd_all, nmr_all, hT,
                 ident_bf16, name, its=None):
    """xn = (x-mean)*rstd -> transpose -> fused (1+scale)/shift copy.

    Tiles are processed in PAIRS: one [128, 1024] xn activation and
    per-b STTs over 256-wide free dims, halving the per-op overhead that
    used to dominate DVE time in this phase.
    """
    with contextlib.ExitStack() as actx:
        xn_pool = actx.enter_context(tc.tile_pool(name=f"{name}_xn", bufs=3))
        pst_pool = actx.enter_context(
            tc.tile_pool(name=f"{name}_pst", bufs=2, space="PSUM")
        )
        idx = list(its if its is not None else range(n_tiles))
        for i0 in range(0, len(idx), 2):
            pair = idx[i0:i0 + 2]
            xn = xn_pool.tile([P, 2, 512], bf16, tag="xn")
            for j, it in enumerate(pair):
                nc.scalar.activation(xn[:, j, :], src_tiles(it), AF.Identity,
                                     bias=nmr_all[:, it:it + 1],
                                     scale=rstd_all[:, it:it + 1])
            xnt = pst_pool.tile([P, 2, 4, P], bf16, tag="xnt")
            for j, it in enumerate(pair):
                for b in range(4):
                    nc.tensor.transpose(
                        xnt[:, j, b, :], xn[:, j, b * P:(b + 1) * P], ident_bf16
                    )
            if len(pair) == 2 and pair[1] == pair[0] + 1:
                it = pair[0]
                for b in range(4):
                    nc.vector.tensor_scalar(
                        hT[:, b, it * P:(it + 2) * P], xnt[:, :, b, :],
                        ab[:, b:b + 1], ab[:, 4 + b:5 + b],
                        op0=OP.mult, op1=OP.add,
                    )
            else:
                for j, it in enumerate(pair):
                    for b in range(4):
                        nc.vector.tensor_scalar(
                            hT[:, b, it * P:(it + 1) * P], xnt[:, j, b, :],
                            ab[:, b:b + 1], ab[:, 4 + b:5 + b],
                            op0=OP.mult, op1=OP.add,
                        )


def _adaln_to_hT(nc, tc, src_tiles, n_tiles, ab, hT, ident_bf16, eps_sb, name):
    with contextlib.ExitStack() as actx:
        stat_pool = actx.enter_context(tc.tile_pool(name=f"{name}_stat", bufs=4))
        rstd_all, nmr_all = _adaln_stats(nc, stat_pool, src_tiles, n_tiles,
                                         eps_sb)
        _adaln_apply(nc, tc, src_tiles, n_tiles, ab, rstd_all, nmr_all, hT,
                     ident_bf16, name)


VW = HD + 1       # 65: per-head v window [v_h(64) | 1]
VROW = VW * H     # 520: all 8 head windows of one key tile, contiguous


def _vc_memset(nc, vC, njt):
    nc.vector.memset(
        vC[:, 0:njt * VROW].rearrange("p (j h w) -> p j h w", j=njt, h=H)[:, :, :, HD:VW],
        1.0,
    )
    nc.vector.memset(vC[:, njt * VROW:], 0.0)


def _vc_copy(nc, vC, jt, ps):
    """psum [128, 512] (8 heads x 64) -> vC head windows; on ScalarE to keep
    DVE free for the adaln applies that share this phase."""
    dst = vC[:, jt * VROW:(jt + 1) * VROW].rearrange("p (h w) -> p h w", h=H)
    nc.scalar.copy(dst[:, :, 0:HD], ps.rearrange("p (h d) -> p h d", h=H))


def _flush_av(nc, vC, njt, pend, avps, pools, ones_row, av_all):
    ht, et, jt = pend
    ps_s, et_pool, dn_pool = pools
    avp = avps[ht]
    for s in range(2):
        h = 2 * ht + s
        nc.tensor.matmul(
            avp[:, s, :], vC[:, jt * VROW + VW * h:jt * VROW + VW * h + P],
            et[:, s, :],
            start=(jt == 0), stop=(jt == njt - 1),
        )
    if jt == njt - 1:
        # pair finished: softmax tail. Both slots' denominators sit in psum
        # row 64 of the pair tile; copy to SBUF bf16, K=1 broadcast matmuls
        # of the RAW denominators into a [128, 512] psum, drain to f32, one
        # [128,512] fast approx reciprocal (512 elems/lane), then scale.
        dnm = dn_pool.tile([1, 2, ROWS], bf16, tag="dnm")
        nc.vector.tensor_copy(dnm, avp[HD:HD + 1, :, :])
        rb = ps_s.tile([P, 2, ROWS], f32, tag="s")
        for s in range(2):
            nc.tensor.matmul(
                rb[s * HD:(s + 1) * HD, 0, :],
                ones_row[0:1, 0:HD],
                dnm[0:1, s, :],
                start=True, stop=True,
            )
        rb_f = dn_pool.tile([P, ROWS], f32, tag="rbf")
        nc.vector.tensor_copy(rb_f, rb[:, 0, :])
        rb_r = dn_pool.tile([P, ROWS], f32, tag="rbr")
        nc.vector.reciprocal_approx_fast(rb_r, rb_f)
        for s in range(2):
            po = 64 * s
            nc.vector.scalar_tensor_tensor(
                av_all[po:po + HD, ht, :],
                avp[0:HD, s, :], 1.0, rb_r[po:po + HD, :],
                op0=OP.mult, op1=OP.mult,
            )


def _attention(nc, tc, act, qz, kT, vC, njt, wo, ob_row, ones_row,
               x_res, x_out, name):
    """Transposed-score attention for 8 heads over own 512 rows.

    qz: [128, 4 ht, 2, rows]; kT: [128, 4, keys]; vC: [128, njt*520+63].
    All attention matmuls are full 128x128-array (keeps the HAM clock
    gate open). Writes x_out = attn_out @ wo + ob + x_res.
    """
    av_all = act.tile([P, 4, ROWS], bf16, tag="tH")
    with (
        tc.tile_pool(name=f"{name}_ps_s", bufs=2, space="PSUM") as ps_s,
        tc.tile_pool(name=f"{name}_ps_av", bufs=2, space="PSUM") as ps_av,
        tc.tile_pool(name=f"{name}_et", bufs=3) as et_pool,
        tc.tile_pool(name=f"{name}_dn", bufs=1) as dn_pool,
    ):
        pools = (ps_s, et_pool, dn_pool)
        avps = {}
        # software-pipelined: scores/exp for step n+1 issue before the
        # av matmuls of step n, so the in-order PE stream never waits on
        # ScalarE's exp latency. The skew also crosses pair boundaries.
        pend = None   # (ht, et, jt)
        for ht in range(4):
            avps[ht] = ps_av.tile([P, 2, ROWS], f32, tag="av", name="avp")
            for jt in range(njt):
                sps = ps_s.tile([P, 2, ROWS], f32, tag="s")
                for s in range(2):
                    nc.tensor.matmul(
                        sps[:, s, :],
                        kT[:, ht, jt * P:(jt + 1) * P],
                        qz[:, ht, s, :],
                        start=True, stop=True,
                    )
                et = et_pool.tile([P, 2, ROWS], bf16, tag="et")
                nc.scalar.activation(et, sps, AF.Exp, scale=SCALE)
                if pend is not None:
                    _flush_av(nc, vC, njt, pend, avps, pools, ones_row, av_all)
                pend = (ht, et, jt)
            # flush at pair end is deferred; pend carries over
        if pend is not None:
            _flush_av(nc, vC, njt, pend, avps, pools, ones_row, av_all)
    # out-projection + bias + residual
    with tc.tile_pool(name=f"{name}_ps_o", bufs=2, space="PSUM") as ps_o:
        for it in range(4):
            ps = ps_o.tile([P, D], f32, tag="o")
            for dt_ in range(4):
                nc.tensor.matmul(
                    ps, av_all[:, dt_, it * P:(it + 1) * P], wo[:, dt_, :],
                    start=(dt_ == 0), stop=False,
                )
            nc.tensor.matmul(
                ps, ones_row[0:1, 0:P], ob_row, start=False, stop=True,
            )
            nc.vector.tensor_tensor(x_out[:, it, :], ps, x_res[:, it, :], op=OP.add)
    return av_all


def build(max_phase=5, debug=False):
    nc = bacc.Bacc(None, target_bir_lowering=False)

    # ---------------- I/O (host-prepacked layouts) ----------------
    xo_d = nc.dram_tensor("xo", [P, 4, D], f32, kind="ExternalInput")
    xr_d = nc.dram_tensor("xr", [P, 12, D], bf16, kind="ExternalInput")
    condT_d = nc.dram_tensor("condT", [P, 4, NCTX], bf16, kind="ExternalInput")
    tT_d = nc.dram_tensor("tT", [P, 4], bf16, kind="ExternalInput")
    nw = {}
    nb_ = {}
    for l in (1, 2, 4):
        nw[l] = nc.dram_tensor(f"n{l}_w", [P, 4, 2 * D], bf16, kind="ExternalInput")
        nb_[l] = nc.dram_tensor(f"n{l}_b", [P, 8], f32, kind="ExternalInput")
    aw = {}
    for a in (1, 2):
        aw[a] = nc.dram_tensor(f"a{a}_w", [P, 4, 4, D], bf16, kind="ExternalInput")
        aw[a, "ob"] = nc.dram_tensor(f"a{a}_ob", [1, D], bf16, kind="ExternalInput")
    w1a_d = nc.dram_tensor("ff_w1a", [P, 4, 4 * D], bf16, kind="ExternalInput")
    w1b_d = nc.dram_tensor("ff_w1b", [P, 4, 4 * D], bf16, kind="ExternalInput")
    w2_d = nc.dram_tensor("ff_w2", [P, 16, D], bf16, kind="ExternalInput")
    b1_d = nc.dram_tensor("ff_b1", [P, 32], f32, kind="ExternalInput")
    b2_d = nc.dram_tensor("ff_b2", [1, D], bf16, kind="ExternalInput")
    out = nc.dram_tensor("out", [ROWS, D], f32, kind="ExternalOutput")

    with tile.TileContext(nc) as tc, contextlib.ExitStack() as ctx:
        const = ctx.enter_context(tc.tile_pool(name="const", bufs=1))
        wpool = ctx.enter_context(tc.tile_pool(name="wpool", bufs=1))
        act = ctx.enter_context(tc.tile_pool(name="act", bufs=1))

        ident_bf16 = const.tile([P, P], bf16)
        make_identity(nc, ident_bf16)
        ident_f32 = const.tile([P, P], f32)
        make_identity(nc, ident_f32)
        ones_row = const.tile([1, P], bf16)
        nc.vector.memset(ones_row, 1.0)
        eps_sb = const.tile([P, 1], f32)
        nc.vector.memset(eps_sb, EPS)

        # PE warmup: dependency-free matmuls fill the otherwise idle
        # startup window and lift the HAM clock gate to 2.4 GHz early
        with tc.tile_pool(name="warm", bufs=1, space="PSUM") as warm_pool:
            wps = warm_pool.tile([P, P], f32)
            for _ in range(50):
                nc.tensor.matmul(wps, ident_bf16, ident_bf16,
                                 start=True, stop=True)

        # ------- input DMAs, all contiguous HWDGE, in priority order -----
        tT = const.tile([P, 4], bf16)
        nc.sync.dma_start(tT, tT_d[:])
        nw1_sb = wpool.tile([P, 4, 2 * D], bf16, tag="nw1")
        nc.sync.dma_start(nw1_sb, nw[1][:])
        nb1_col = const.tile([P, 8], f32, tag="nb1")
        nc.sync.dma_start(nb1_col, nb_[1][:])

        own_x = act.tile([P, 4, D], f32, tag="tE")
        x_tiles = {}
        for it in range(4):
            nc.sync.dma_start(own_x[:, it, :], xo_d[:][:, it, :])
            x_tiles[it] = own_x[:, it, :]
        a_sb = {}
        stack1 = wpool.tile([P, 4, 4, D], bf16, tag="wbig1")
        nc.sync.dma_start(stack1, aw[1][:])
        for wi, w in enumerate("qkvo"):
            a_sb[1, w] = stack1[:, :, wi, :]
        xrest = act.tile([P, 12, D], bf16, tag="tX")
        for c in range(3):
            nc.sync.dma_start(xrest[:, c * 4:(c + 1) * 4, :],
                              xr_d[:][:, c * 4:(c + 1) * 4, :])
            for it in range(4):
                x_tiles[4 + c * 4 + it] = xrest[:, c * 4 + it, :]
        condT = act.tile([P, 4, NCTX], bf16, tag="tE2")
        nc.sync.dma_start(condT, condT_d[:])
        stack2 = wpool.tile([P, 4, 4, D], bf16, tag="wbig2")
        nc.sync.dma_start(stack2, aw[2][:])
        for wi, w in enumerate("qkvo"):
            a_sb[2, w] = stack2[:, :, wi, :]
        for a in (1, 2):
            ob = wpool.tile([1, D], bf16, tag=f"a{a}ob")
            nc.sync.dma_start(ob, aw[a, "ob"][:])
            a_sb[a, "ob"] = ob

        # norm weights for layers 2/4 + FFN weights ride the gpsimd queue
        # so the sync queue's critical-path loads are never behind them.
        nw_sb = {1: nw1_sb}
        nb_col = {1: nb1_col}
        for l in (2, 4):
            nw_sb[l] = wpool.tile([P, 4, 2 * D], bf16, tag=f"nw{l}",
                                  name=f"nw{l}_sb")
            nc.gpsimd.dma_start(nw_sb[l], nw[l][:])
            nb_col[l] = const.tile([P, 8], f32, tag=f"nb{l}",
                                   name=f"nb{l}_col")
            nc.gpsimd.dma_start(nb_col[l], nb_[l][:])

        # ---------------- norm scale/shift params ----------------
        def _emb(l):
            with (
                tc.tile_pool(name=f"embp{l}", bufs=1) as embp,
                tc.tile_pool(name=f"ps_emb{l}", bufs=2, space="PSUM") as ps_emb,
            ):
                emb_ps = ps_emb.tile([1, 2 * D], f32, tag="embps")
                for half in range(2):
                    for kt in range(4):
                        nc.tensor.matmul(
                            emb_ps[:, half * D:(half + 1) * D],
                            tT[:, kt:kt + 1],
                            nw_sb[l][:, kt, half * D:(half + 1) * D],
                            start=(kt == 0), stop=(kt == 3),
                        )
                emb_row = embp.tile([1, 2 * D], f32, tag="embrow")
                nc.vector.tensor_copy(emb_row, emb_ps)
                # row -> per-partition columns via tiny PE transposes;
                # bias column add + the +1 for scale cols fused into the
                # psum drain
                ab_l = const.tile([P, 8], f32, tag=f"ab{l}")
                for col in range(8):
                    tp = ps_emb.tile([P, 1], f32, tag="embT")
                    nc.tensor.transpose(
                        tp, emb_row[0:1, col * P:(col + 1) * P],
                        ident_f32[0:1, 0:1]
                    )
                    nc.vector.scalar_tensor_tensor(
                        ab_l[:, col:col + 1], tp,
                        1.0 if col < 4 else 0.0, nb_col[l][:, col:col + 1],
                        op0=OP.add, op1=OP.add,
                    )
            return ab_l

        ab = {1: _emb(1)}

        h1T = act.tile([P, 4, NB], bf16, tag="tA")

        # adaln1 stats issue first: independent of norm weights, keeps DVE
        # busy while the emb chain waits on its weight DMAs
        n1_stat = ctx.enter_context(tc.tile_pool(name="n1_stat", bufs=4))
        if max_phase >= 1:
            rstd1, nmr1 = _adaln_stats(nc, n1_stat, lambda it: x_tiles[it],
                                       16, eps_sb, groups=[(0, 4), (4, 16)])

        # ---------------- phase 2: projections q1z, k1T, vC1 -------------
        # Apply tiles 0-3 (own rows) first so the q projection starts as
        # early as possible; k/v follow as the remaining applies complete.
        if max_phase >= 2:
            k1T = act.tile([P, 4, NB], bf16, tag="tB")
            vC1 = act.tile([P, 16 * VROW + HD - 1], bf16, tag="tC")
            _vc_memset(nc, vC1, 16)
            q1z = act.tile([P, 4, 2, ROWS], bf16, tag="tD")
            nc.vector.memset(q1z[HD:P, :, 0, :], 0.0)
            nc.vector.memset(q1z[0:HD, :, 1, :], 0.0)

            def _kproj(ps_k, kT_dst, h_src, wkey, dt_, jc):
                # two N=512 matmuls cover 1024 keys into one [P,2,512]
                # psum tile; a single copy drains both halves
                ps = ps_k.tile([P, 2, 512], f32, tag="projk")
                for half in range(2):
                    for kt in range(4):
                        nc.tensor.matmul(
                            ps[:, half, :],
                            a_sb[wkey][:, kt, dt_ * P:(dt_ + 1) * P],
                            h_src[:, kt, jc * 1024 + half * 512:
                                  jc * 1024 + (half + 1) * 512],
                            start=(kt == 0), stop=(kt == 3),
                        )
                nc.vector.tensor_copy(
                    kT_dst[:, dt_, jc * 1024:(jc + 1) * 1024].rearrange(
                        "p (a b) -> p a b", a=2),
                    ps,
                )

            def _vproj(ps_qv, vC_dst, h_src, wkey, jt):
                ps = ps_qv.tile([P, 512], f32, tag="proj")
                for kt in range(4):
                    nc.tensor.matmul(
                        ps,
                        h_src[:, kt, jt * P:(jt + 1) * P],
                        a_sb[wkey][:, kt, :],
                        start=(kt == 0), stop=(kt == 3),
                    )
                _vc_copy(nc, vC_dst, jt, ps)

            def _qproj(ps_qv, qz_dst, hT_src, wkey):
                for dt_ in range(4):
                    ps = ps_qv.tile([P, 512], f32, tag="proj")
                    for kt in range(4):
                        nc.tensor.matmul(
                            ps,
                            a_sb[wkey][:, kt, dt_ * P:(dt_ + 1) * P],
                            hT_src[:, kt, 0:ROWS],
                            start=(kt == 0), stop=(kt == 3),
                        )
                    nc.vector.tensor_copy(qz_dst[0:HD, dt_, 0, :], ps[0:HD, :])
                    nc.vector.tensor_copy(qz_dst[HD:P, dt_, 1, :], ps[HD:P, :])

            _adaln_apply(nc, tc, lambda it: x_tiles[it], 16, ab[1], rstd1,
                         nmr1, h1T, ident_bf16, "n1a", its=range(0, 4))
            with (
                tc.tile_pool(name="ps_k1", bufs=2, space="PSUM") as ps_k,
                tc.tile_pool(name="ps_qv1", bufs=2, space="PSUM") as ps_qv,
            ):
                _qproj(ps_qv, q1z, h1T, (1, "q"))
                _adaln_apply(nc, tc, lambda it: x_tiles[it], 16, ab[1], rstd1,
                             nmr1, h1T, ident_bf16, "n1b", its=range(4, 8))
                for dt_ in range(4):
                    _kproj(ps_k, k1T, h1T, (1, "k"), dt_, 0)
                for jt in range(0, 8):
                    _vproj(ps_qv, vC1, h1T, (1, "v"), jt)
                _adaln_apply(nc, tc, lambda it: x_tiles[it], 16, ab[1], rstd1,
                             nmr1, h1T, ident_bf16, "n1c", its=range(8, 16))
                for dt_ in range(4):
                    _kproj(ps_k, k1T, h1T, (1, "k"), dt_, 1)
                for jt in range(8, 16):
                    _vproj(ps_qv, vC1, h1T, (1, "v"), jt)

        if debug and max_phase >= 2:
            dbg_h1 = nc.dram_tensor("dbg_h1", [P, 4, NB], bf16, kind="ExternalOutput")
            nc.sync.dma_start(dbg_h1[:], h1T)
            dbg_k1 = nc.dram_tensor("dbg_k1", [P, 4, NB], bf16, kind="ExternalOutput")
            nc.sync.dma_start(dbg_k1[:], k1T)
            dbg_q1 = nc.dram_tensor("dbg_q1", [P, 4, 2, ROWS], bf16,
                                    kind="ExternalOutput")
            nc.sync.dma_start(dbg_q1[:], q1z)
            dbg_v1 = nc.dram_tensor("dbg_v1", [P, 16 * VROW + HD - 1], bf16,
                                    kind="ExternalOutput")
            nc.sync.dma_start(dbg_v1[:], vC1)

        # deferred adaln2 params: PE work lands in the proj -> att1 seam
        ab[2] = _emb(2)

        # ---------------- phase 3: attention 1 ---------------------------
        if max_phase >= 3:
            x2 = act.tile([P, 4, D], f32, tag="tF")
            av_dbg = _attention(nc, tc, act, q1z, k1T, vC1, 16, a_sb[1, "o"],
                                a_sb[1, "ob"], ones_row, own_x, x2, "att1")
            if debug:
                dbg_av = nc.dram_tensor("dbg_av", [P, 4, ROWS], bf16,
                                        kind="ExternalOutput")
                nc.sync.dma_start(dbg_av[:], av_dbg)
            final = x2
        else:
            final = own_x

        # FFN weights: issued here so their DMAs run during attention-1.
        # w1 halves live in the dead h1T / vC1 slots (WAR deps handle it).
        if max_phase >= 5:
            w1a = act.tile([P, 4, 4 * D], bf16, tag="tA")
            nc.gpsimd.dma_start(w1a, w1a_d[:])
            w1b = act.tile([P, 4, 4 * D], bf16, tag="tC")
            nc.gpsimd.dma_start(w1b, w1b_d[:])
            w2_sb = wpool.tile([P, 16, D], bf16, tag="wff2")
            nc.gpsimd.dma_start(w2_sb, w2_d[:])
            b1_sb = const.tile([P, 32], f32)
            nc.gpsimd.dma_start(b1_sb, b1_d[:])
            b2_row = const.tile([1, D], bf16)
            nc.gpsimd.dma_start(b2_row, b2_d[:])

        ab[4] = _emb(4)

        # ------- seam: adaln2 (DVE/Scalar) runs while the PE does the ----
        # ------- cross-attn K/V projections (independent of x2) ----------
        if max_phase >= 4:
            h2T = act.tile([P, 4, ROWS], bf16, tag="tH")
            with contextlib.ExitStack() as sctx:
                stat2 = sctx.enter_context(tc.tile_pool(name="n2_stat", bufs=4))
                rstd2, nmr2 = _adaln_stats(nc, stat2, lambda it: x2[:, it, :],
                                           4, eps_sb)

                k2T = act.tile([P, 4, NCTX], bf16, tag="tX")
                vC2 = act.tile([P, 8 * VROW + HD - 1], bf16, tag="tI")
                _vc_memset(nc, vC2, 8)
                q2z = act.tile([P, 4, 2, ROWS], bf16, tag="tD")
                nc.vector.memset(q2z[HD:P, :, 0, :], 0.0)
                nc.vector.memset(q2z[0:HD, :, 1, :], 0.0)
                with (
                    tc.tile_pool(name="ps_k2", bufs=2, space="PSUM") as ps_k,
                    tc.tile_pool(name="ps_qv2", bufs=2, space="PSUM") as ps_qv,
                ):
                    for dt_ in range(4):
                        _kproj(ps_k, k2T, condT, (2, "k"), dt_, 0)
                    for jt in range(8):
                        _vproj(ps_qv, vC2, condT, (2, "v"), jt)
                    _adaln_apply(nc, tc, lambda it: x2[:, it, :], 4, ab[2],
                                 rstd2, nmr2, h2T, ident_bf16, "n2")
                    _qproj(ps_qv, q2z, h2T, (2, "q"))

            x3 = act.tile([P, 4, D], f32, tag="tG")
            _attention(nc, tc, act, q2z, k2T, vC2, 8, a_sb[2, "o"],
                       a_sb[2, "ob"], ones_row, x2, x3, "att2")
            final = x3

        # ---------------- phase 5: adaln3 + GEGLU FFN --------------------
        if max_phase >= 5:
            h3T = act.tile([P, 4, ROWS], bf16, tag="tD")
            _adaln_to_hT(nc, tc, lambda it: x3[:, it, :], 4, ab[4], h3T,
                         ident_bf16, eps_sb, "n4")

            ugT = act.tile([P, 16, ROWS], bf16, tag="tB")
            with (
                tc.tile_pool(name="ps_z", bufs=4, space="PSUM") as ps_z,
                tc.tile_pool(name="gact", bufs=3) as gact_pool,
            ):
                for ut in range(16):
                    zu = ps_z.tile([P, ROWS], f32, tag="z")
                    zg = ps_z.tile([P, ROWS], f32, tag="z")
                    # zg first: gelu (ScalarE) starts 4 matmuls earlier,
                    # overlapping the zu matmuls
                    for kt in range(4):
                        nc.tensor.matmul(
                            zg, w1b[:, kt, ut * P:(ut + 1) * P],
                            h3T[:, kt, :], start=(kt == 0), stop=(kt == 3),
                        )
                    for kt in range(4):
                        nc.tensor.matmul(
                            zu, w1a[:, kt, ut * P:(ut + 1) * P],
                            h3T[:, kt, :], start=(kt == 0), stop=(kt == 3),
                        )
                    gact = gact_pool.tile([P, ROWS], bf16, tag="gact")
                    nc.scalar.activation(
                        gact, zg, AF.Gelu, bias=b1_sb[:, 16 + ut:17 + ut], scale=1.0
                    )
                    nc.vector.scalar_tensor_tensor(
                        ugT[:, ut, :], zu, b1_sb[:, ut:ut + 1], gact,
                        op0=OP.add, op1=OP.mult,
                    )

            out_sb = act.tile([P, 4, D], f32, tag="tC")
            with tc.tile_pool(name="ps_y", bufs=2, space="PSUM") as ps_y:
                for it in range(4):
                    ps = ps_y.tile([P, D], f32, tag="y")
                    for kt in range(16):
                        nc.tensor.matmul(
                            ps, ugT[:, kt, it * P:(it + 1) * P],
                            w2_sb[:, kt, :],
                            start=(kt == 0), stop=False,
                        )
                    nc.tensor.matmul(
                        ps, ones_row[0:1, 0:P], b2_row, start=False, stop=True,
                    )
                    nc.vector.tensor_tensor(
                        out_sb[:, it, :], ps, x3[:, it, :], op=OP.add
                    )
                    # drain each row-block while the next one computes
                    nc.sync.dma_start(out[:][it * P:(it + 1) * P, :],
                                      out_sb[:, it, :])
            final = None

        if final is not None:
            for it_ in range(4):
                nc.sync.dma_start(out[:][it_ * P:(it_ + 1) * P, :],
                                  final[:, it_, :])

    nc.compile()
    return nc


def _pack_w(w):
    """[K, N] f32 -> [128, K//128, N] bf16 (d = k*128 + p)."""
    K, Nc = w.shape
    return np.ascontiguousarray(
        w.reshape(K // P, P, Nc).transpose(1, 0, 2).astype(ml_dtypes.bfloat16))


def _shard_inputs(inputs):
    """Build the 8 per-core input maps (prepacked SBUF layouts, bf16)."""
    bf = ml_dtypes.bfloat16
    x = np.ascontiguousarray(inputs["x"], dtype=np.float32)
    t = np.ascontiguousarray(inputs["t"], dtype=np.float32)
    cond = np.ascontiguousarray(inputs["cond"], dtype=np.float32)

    shared = {}
    for l in (1, 2, 4):
        shared[f"n{l}_w"] = _pack_w(inputs[f"n{l}_w"].astype(np.float32))
        shared[f"n{l}_b"] = np.ascontiguousarray(
            inputs[f"n{l}_b"].astype(np.float32).reshape(8, P).T)
    for a in (1, 2):
        stack = np.stack(
            [_pack_w(inputs[f"a{a}_{w}"].astype(np.float32)) for w in "qkvo"],
            axis=2)  # [128, 4, 4, 512]
        shared[f"a{a}_w"] = np.ascontiguousarray(stack)
        shared[f"a{a}_ob"] = np.ascontiguousarray(
            inputs[f"a{a}_ob"].astype(np.float32).reshape(1, D).astype(bf))
    ff_w1 = inputs["ff_w1"].astype(np.float32)
    shared["ff_w1a"] = _pack_w(ff_w1[:, 0:4 * D])
    shared["ff_w1b"] = _pack_w(ff_w1[:, 4 * D:8 * D])
    shared["ff_w2"] = _pack_w(inputs["ff_w2"].astype(np.float32))
    shared["ff_b1"] = np.ascontiguousarray(
        inputs["ff_b1"].astype(np.float32).reshape(32, P).T)
    shared["ff_b2"] = np.ascontiguousarray(
        inputs["ff_b2"].astype(np.float32).reshape(1, D).astype(bf))

    in_maps = []
    for c in range(NCORES):
        b = c // 4
        r0 = (c % 4) * ROWS
        m = dict(shared)
        xr_full = np.roll(x[b], -r0, axis=0)          # [2048, 512]
        m["xo"] = np.ascontiguousarray(
            xr_full[0:ROWS].reshape(4, P, D).transpose(1, 0, 2))
        m["xr"] = np.ascontiguousarray(
            xr_full[ROWS:].reshape(12, P, D).transpose(1, 0, 2).astype(bf))
        # condT[p, kt, j] = cond[b, j, kt*128+p]
        m["condT"] = np.ascontiguousarray(
            cond[b].reshape(NCTX, 4, P).transpose(2, 1, 0).astype(bf))
        m["tT"] = np.ascontiguousarray(
            t[b, 0].reshape(4, P).T.astype(bf))
        in_maps.append(m)
    return in_maps


def kernel(**inputs) -> np.ndarray:
    if "nc" not in _CACHED:
        _CACHED["nc"] = build()
    nc = _CACHED["nc"]
    in_maps = _shard_inputs(inputs)
    res = run_bass_kernel_spmd(nc, in_maps, core_ids=list(range(NCORES)))
    outs = [res.results[c]["out"] for c in range(NCORES)]
    full = np.concatenate(outs, axis=0).reshape(B, N, D)
    return full.astype(np.float32)
